# revision 48
# baseline (speedup 1.0000x reference)
"""Trainium2 Bass kernel for AttentionWithRotaryPosEmb (8 cores, data-parallel).

Strategy
--------
Data-parallel over batch: each of the 8 NeuronCores computes one batch element
end-to-end. No collectives needed.

Per-core pipeline (batch x_b is [C=256, S=1024]):
  1. QKV projection: q,k as [o, s] (o = 64h+d), vT as [s, hid] straight out of
     the matmuls -- no transposes needed on-chip. PSUM evacuated by ACT copy
     (f32 -> bf16) so DVE stays free and all later elementwise runs in bf16
     DVE perf modes (2x/4x).
  2. RoPE on q,k in [d, s] layout: rotate-half is a single DVE stream_shuffle
     (32-partition-group permutation), then sin/cos multiplies + add against
     host-precomputed row tables (sign of sin encodes rotate-half; rows with
     d >= 32 have sin=0, cos=1).
  3. L2 norm over the sequence axis: fused square+reduce on DVE
     (tensor_tensor_reduce), rsqrt via exp(-0.5*ln(x)) on ACT (same activation
     table set as the softmax exp -> no table reloads). Both q and k scales
     fold into q.
  4. Attention with transposed softmax: simT[j, i] via matmul(lhsT=k_h,
     rhs=q_h); exp on ACT with scale=10 fused; softmax denominators come free
     from a ones-column appended to vT (65th lhsT column); outT[d, i]
     accumulates over j-blocks in PSUM.
  5. Softmax normalization: the denominator row is evacuated and reshaped via
     DMA to a [16, 64] layout so the DVE reciprocal runs multi-partition
     (0.5us instead of 6.5us single-lane), DMA'd back to a row, broadcast
     across partitions on the (otherwise idle) GPSIMD engine, then one DVE
     multiply. All shuttle DMAs issue from the GPSIMD queue, keeping the Sync
     engine out of the critical path.
  6. Output projection with the bias applied by ACT (per-partition bias AP).
"""

import os
import sys

import numpy as np

if "/opt/trn_rl_repo" not in sys.path:
    sys.path.insert(0, "/opt/trn_rl_repo")

# bisect fallbacks (baseline-proven implementations).  tensor_tensor_reduce
# crashes at runtime on this hw/runtime combo -> always use mul+reduce_sum.
NO_SHUFFLE = os.environ.get("K_NO_SHUFFLE") == "1"
NO_TTR = True
NO_IDBIAS = os.environ.get("K_NO_IDBIAS") == "1"
NO_BCASTMM = os.environ.get("K_NO_BCASTMM") == "1"

HEADS = 8
DH = 64
S = 1024
C = 256
HID = 512
ROT = 32
HALF = 16
SCALE = 10.0
N_CORES = 8

# rotate-half as a 32-partition-group shuffle: swap the 16-row halves
SHUF_MASK = list(range(16, 32)) + list(range(16))

_CACHE = {}


def _rope_tables():
    """Row-patterned cos/sin tables [128, S] matching the q/k SBUF layout.

    Partition row r holds o-row (128t + r) of q/k tile t, i.e. head 2t + (r>=64)
    and d = r % 64.  Rows d in [0,16) get cos(i*invf[d]) / -sin(i*invf[d]);
    rows d in [16,32) get cos(i*invf[d-16]) / +sin(i*invf[d-16]); rows d >= 32
    get cos=1, sin=0 (identity).  The sign of sin encodes rotate_half.
    """
    inv = (
        1.0 / (np.float32(10000.0) ** (np.arange(0, ROT, 2, dtype=np.float32) / np.float32(ROT)))
    ).astype(np.float32)
    ang = (np.arange(S, dtype=np.float32)[None, :] * inv[:, None]).astype(np.float32)
    cos16 = np.cos(ang).astype(np.float32)  # [16, S]
    sin16 = np.sin(ang).astype(np.float32)
    cosT = np.ones((128, S), np.float32)
    sinT = np.zeros((128, S), np.float32)
    for r in range(128):
        d = r % 64
        if d < HALF:
            cosT[r] = cos16[d]
            sinT[r] = -sin16[d]
        elif d < ROT:
            cosT[r] = cos16[d - HALF]
            sinT[r] = sin16[d - HALF]
    return cosT, sinT


def _emit(ctx, tc, aps):
    import concourse.bass as bass  # noqa: F401
    from concourse import mybir

    f32 = mybir.dt.float32
    bf16 = mybir.dt.bfloat16
    f8 = mybir.dt.float8e4
    DR = mybir.MatmulPerfMode.DoubleRow
    AF = mybir.ActivationFunctionType
    ALU = mybir.AluOpType
    nc = tc.nc
    x_d, wqk_d, wv_d, wo_d, bo_d, cos_d, sin_d, out_d = aps

    singles = ctx.enter_context(tc.tile_pool(name="singles", bufs=1))
    wk = ctx.enter_context(tc.tile_pool(name="wk", bufs=3))
    ppm = ctx.enter_context(tc.tile_pool(name="ppm", bufs=2, space="PSUM"))
    ppo = ctx.enter_context(tc.tile_pool(name="ppo", bufs=2, space="PSUM"))

    # ---- persistent SBUF ----
    sb_x = singles.tile([128, 2, S], bf16)
    sb_wqk = singles.tile([128, 2, 2 * HID], bf16)
    sb_wv = singles.tile([128, 2, HID], bf16)
    sb_wo = singles.tile([128, 4, C], bf16)
    sb_bo = singles.tile([128, 2], f32)
    sb_cos = singles.tile([128, S], bf16)
    sb_sin = singles.tile([128, S], bf16)
    sb_q = singles.tile([128, 4, S], bf16)   # raw q -> roped q (pre-scale)
    sb_k = singles.tile([128, 4, S], bf16)   # raw k (k*cos staging in-place)
    sb_qb = singles.tile([128, 4, S], bf16)  # normalized q, sim operand
    sb_kb = singles.tile([128, 4, S], bf16)  # roped k, sim operand
    sb_vx = singles.tile([128, 8, HEADS * (DH + 1)], bf16)
    sb_or = singles.tile([128, 4, S], bf16)  # attention out, unnormalized
    sb_o = singles.tile([128, 4, S], bf16)   # attention out, proj operand
    sb_stat = singles.tile([128, 16], f32)
    den_sh = singles.tile([128, 8, 8], f32)     # partition-spread denominators
    rcp_sh = singles.tile([128, 8, 8], bf16)    # reciprocals (bf16: bcast operand)
    ones_col = singles.tile([1, 64], bf16)      # lhsT for the rcp row broadcast
    acc = singles.tile([128, 2, S], f32)        # output-projection accumulator

    # ---- input loads (c-block granularity so QKV matmuls can start early) ----
    nc.sync.dma_start(out=sb_wqk[:, 0, :], in_=wqk_d[0:128, :])
    nc.sync.dma_start(out=sb_x[:, 0, :], in_=x_d[0:128, :])
    nc.sync.dma_start(out=sb_wqk[:, 1, :], in_=wqk_d[128:256, :])
    nc.sync.dma_start(out=sb_x[:, 1, :], in_=x_d[128:256, :])
    nc.sync.dma_start(out=sb_wv[:, 0, :], in_=wv_d[0:128, :])
    nc.sync.dma_start(out=sb_wv[:, 1, :], in_=wv_d[128:256, :])
    nc.sync.dma_start(out=sb_cos[:, :], in_=cos_d[:, :])
    nc.sync.dma_start(out=sb_sin[:, :], in_=sin_d[:, :])
    for kk in range(4):
        nc.sync.dma_start(out=sb_wo[:, kk, :], in_=wo_d[kk * 128 : (kk + 1) * 128, :])
    nc.sync.dma_start(out=sb_bo[:, :], in_=bo_d[:, :])

    # ones column per head in vT_ext -> free softmax denominators
    vx4 = sb_vx.rearrange("p j (h e) -> p j h e", e=DH + 1)
    nc.vector.memset(vx4[:, :, :, DH : DH + 1], 1.0)
    nc.vector.memset(ones_col[:, :], 1.0)

    def qk_block(ob):
        """q (ob<4) / k (ob>=4) projection o-block -> SBUF bf16 via ACT."""
        ps = ppm.tile([128, S], f32, tag="mm", name=f"ps_qk{ob}")
        for nn in range(2):
            for kk in range(2):
                nc.tensor.matmul(
                    ps[:, nn * 512 : (nn + 1) * 512],
                    lhsT=(sb_wqk[:, kk, ob * 128 : (ob + 1) * 128]),
                    rhs=(sb_x[:, kk, nn * 512 : (nn + 1) * 512]),
                    start=(kk == 0),
                    stop=(kk == 1),
                )
        dst3 = sb_q if ob < 4 else sb_k
        nc.scalar.copy(out=dst3[:, ob % 4, :], in_=ps[:, :])

    def rope_norm(t):
        """RoPE + l2-norm stats for q/k tile t; fold both scales into q."""
        for src3, col in ((sb_q, 0), (sb_k, 4)):
            cur = src3[:, t, :]
            rot = wk.tile([128, S], bf16, tag="rot", name=f"rot{t}_{col}", bufs=2)
            # rotate-half: swap 16-row halves of each 32-partition group
            if NO_SHUFFLE:
                for base in range(0, 128, 32):
                    nc.sync.dma_start(
                        out=rot[base : base + 16, :], in_=src3[base + 16 : base + 32, t, :]
                    )
                    nc.sync.dma_start(
                        out=rot[base + 16 : base + 32, :], in_=src3[base : base + 16, t, :]
                    )
            else:
                nc.vector.stream_shuffle(out=rot[:, :], in_=cur, mask=SHUF_MASK)
            nc.vector.tensor_mul(out=rot[:, :], in0=rot[:, :], in1=sb_sin[:, :])
            nc.vector.tensor_mul(out=cur, in0=cur, in1=sb_cos[:, :])
            adddst = cur if col == 0 else sb_kb[:, t, :]
            nc.vector.tensor_add(out=adddst, in0=cur, in1=rot[:, :])
            # fused square + sum over s -> stat col
            sq = wk.tile([128, S], bf16, tag="sq", name=f"sq{t}_{col}", bufs=2)
            if NO_TTR:
                nc.vector.tensor_mul(out=sq[:, :], in0=adddst, in1=adddst)
                nc.vector.reduce_sum(
                    out=sb_stat[:, col + t : col + t + 1], in_=sq[:, :],
                    axis=mybir.AxisListType.X,
                )
            else:
                nc.vector.tensor_tensor_reduce(
                    out=sq[:, :], in0=adddst, in1=adddst, scale=1.0, scalar=0.0,
                    op0=ALU.mult, op1=ALU.add,
                    accum_out=sb_stat[:, col + t : col + t + 1],
                )
        # rs_comb = (ssq_q * ssq_k)^-1/2 = exp(-0.5*(ln q + ln k)); the x256
        # from the fp8 qk weights cancels (sim is scale-invariant through it)
        nc.scalar.activation(
            out=sb_stat[:, 8 + t : 9 + t], in_=sb_stat[:, t : t + 1],
            func=AF.Ln, bias=0.0,
        )
        nc.scalar.activation(
            out=sb_stat[:, 12 + t : 13 + t], in_=sb_stat[:, 4 + t : 5 + t],
            func=AF.Ln, bias=0.0,
        )
        nc.vector.tensor_add(
            out=sb_stat[:, 8 + t : 9 + t],
            in0=sb_stat[:, 8 + t : 9 + t],
            in1=sb_stat[:, 12 + t : 13 + t],
        )
        nc.scalar.activation(
            out=sb_stat[:, 12 + t : 13 + t], in_=sb_stat[:, 8 + t : 9 + t],
            func=AF.Exp, scale=-0.5,
        )
        nc.vector.tensor_scalar_mul(
            out=sb_qb[:, t, :], in0=sb_q[:, t, :],
            scalar1=sb_stat[:, 12 + t : 13 + t],
        )

    def v_block(jb):
        """vT s-block: [s_jb, hid] straight from matmul, strided into vx ext."""
        psv = ppo.tile([128, 512], f32, tag="ov", name=f"psv{jb}")
        for kk in range(2):
            nc.tensor.matmul(
                psv[:, :],
                lhsT=(sb_x[:, kk, jb * 128 : (jb + 1) * 128]),
                rhs=(sb_wv[:, kk, :]),
                start=(kk == 0),
                stop=(kk == 1),
            )
        dst = sb_vx[:, jb, :].rearrange("p (h e) -> p h e", e=DH + 1)[:, :, 0:DH]
        src = psv.rearrange("p (h d) -> p h d", d=DH)
        nc.vector.tensor_copy(out=dst, in_=src)  # f32 PSUM -> bf16 SBUF cast

    def attn_head(h):
        t, half = h // 2, h % 2
        b0 = 64 * half
        q_h = sb_qb[b0 : b0 + 64, t, :]
        k_h = sb_kb[b0 : b0 + 64, t, :]
        pso = ppo.tile([65, S], f32, tag="ov", name=f"pso{h}")
        for jb in range(8):
            pss = ppm.tile([128, S], f32, tag="mm", name=f"pss{h}_{jb}")
            for nn in range(2):
                nc.tensor.matmul(
                    pss[:, nn * 512 : (nn + 1) * 512],
                    lhsT=(k_h[:, jb * 128 : (jb + 1) * 128]),
                    rhs=(q_h[:, nn * 512 : (nn + 1) * 512]),
                    start=True,
                    stop=True,
                )
            et = wk.tile([128, S], bf16, tag="et", name=f"et{h}_{jb}", bufs=3)
            nc.scalar.activation(out=et[:, :], in_=pss[:, :], func=AF.Exp, scale=SCALE)
            for nn in range(2):
                nc.tensor.matmul(
                    pso[0:65, nn * 512 : (nn + 1) * 512],
                    lhsT=(sb_vx[:, jb, h * (DH + 1) : (h + 1) * (DH + 1)]),
                    rhs=(et[:, nn * 512 : (nn + 1) * 512]),
                    start=(jb == 0),
                    stop=(jb == 7),
                )
        # park the unnormalized output + denominator row; frees the PSUM buf
        # without waiting for the reciprocal roundtrip
        nc.vector.tensor_copy(out=sb_or[b0 : b0 + 64, t, :], in_=pso[0:64, :])
        dstage = wk.tile([1, S], f32, tag="dstage", name=f"dstage{h}", bufs=2)
        nc.vector.tensor_copy(out=dstage[:, :], in_=pso[64:65, :])
        nc.sync.dma_start(out=den_sh[:, h, :], in_=dstage[:, :])
        with nc.allow_low_precision("softmax denominator reciprocal in bf16"):
            nc.vector.reciprocal(out=rcp_sh[:, h, :], in_=den_sh[:, h, :])

    def attn_norm(h):
        """Broadcast 1/den across partitions via a K=1 PE matmul, then scale.

        Emitted one head late so the PE queue never stalls on the reciprocal
        roundtrip; the broadcast tile borrows a ppm slot (PSUM is full during
        attention).
        """
        t, half = h // 2, h % 2
        b0 = 64 * half
        if NO_BCASTMM:
            # both-SBUF tensor_tensor needs equal base partitions: use a full
            # [128, S] tile and fill rows b0..b0+64 via the doubling chain
            rbc = wk.tile([128, S], bf16, tag="rbc", name=f"rbc{h}", bufs=2)
            nc.sync.dma_start(out=rbc[b0 : b0 + 1, :], in_=rcp_sh[:, h, :])
            w = 1
            while w < 64:
                nc.sync.dma_start(
                    out=rbc[b0 + w : b0 + 2 * w, :], in_=rbc[b0 : b0 + w, :]
                )
                w *= 2
            nc.vector.tensor_mul(
                out=sb_o[b0 : b0 + 64, t, :],
                in0=sb_or[b0 : b0 + 64, t, :],
                in1=rbc[b0 : b0 + 64, :],
            )
            return
        rrow = wk.tile([1, S], bf16, tag="rrow", name=f"rrow{h}", bufs=2)
        nc.sync.dma_start(out=rrow[:, :], in_=rcp_sh[:, h, :])
        pbc = ppm.tile([64, S], f32, tag="mm", name=f"pbc{h}")
        for nn in range(2):
            nc.tensor.matmul(
                pbc[:, nn * 512 : (nn + 1) * 512],
                lhsT=ones_col[0:1, :],
                rhs=rrow[0:1, nn * 512 : (nn + 1) * 512],
                start=True,
                stop=True,
            )
        nc.vector.tensor_mul(
            out=sb_o[b0 : b0 + 64, t, :],
            in0=sb_or[b0 : b0 + 64, t, :],
            in1=pbc[:, :],
        )

    def proj_t(t):
        """Output-projection partial for attention tile t, accumulated into
        SBUF f32 so the matmuls overlap attention instead of serializing at
        the tail.  t=0 evacuates via ACT with the bias fused; later t's add
        the PSUM partial on DVE; t=3 streams the result out."""
        for ob in range(2):
            psf = ppm.tile([128, S], f32, tag="mm", name=f"psf{ob}_{t}")
            for nn in range(2):
                nc.tensor.matmul(
                    psf[:, nn * 512 : (nn + 1) * 512],
                    lhsT=(sb_wo[:, t, ob * 128 : (ob + 1) * 128]),
                    rhs=(sb_o[:, t, nn * 512 : (nn + 1) * 512]),
                    start=True,
                    stop=True,
                )
            if t == 0:
                nc.scalar.activation(
                    out=acc[:, ob, :], in_=psf[:, :], func=AF.Identity,
                    bias=sb_bo[:, ob : ob + 1],
                )
            else:
                nc.vector.tensor_add(
                    out=acc[:, ob, :], in0=psf[:, :], in1=acc[:, ob, :]
                )
                if t == 3:
                    nc.sync.dma_start(
                        out=out_d[ob * 128 : (ob + 1) * 128, :], in_=acc[:, ob, :]
                    )

    # ---- emission order == scheduling priority ----
    qk_block(0)
    qk_block(4)
    rope_norm(0)
    for jb in range(8):
        v_block(jb)
    qk_block(1)
    qk_block(5)
    rope_norm(1)
    attn_head(0)
    attn_head(1)
    attn_norm(0)
    qk_block(2)
    qk_block(6)
    rope_norm(2)
    attn_head(2)
    attn_norm(1)
    proj_t(0)
    attn_head(3)
    attn_norm(2)
    qk_block(3)
    qk_block(7)
    rope_norm(3)
    attn_head(4)
    attn_norm(3)
    proj_t(1)
    attn_head(5)
    attn_norm(4)
    attn_head(6)
    attn_norm(5)
    proj_t(2)
    attn_head(7)
    attn_norm(6)
    attn_norm(7)
    proj_t(3)


def _patch_act_tables():
    """Steer the act-table-load pass to one set covering Exp+Ln+Copy.

    The default pass picks the first table set containing each activation
    function, which ping-pongs between exp_and_others and natural_log
    (~2.7us per reload).  Emptying every other set forces all activations
    onto natural_log_exp_and_others -> exactly one load.
    """
    import concourse.bacc as bacc

    if getattr(bacc, "_act_tables_patched", False):
        return
    import concourse.hw_specs as hw_specs

    orig = hw_specs.get_activation_tables

    def patched(arch):
        tables = orig(arch)
        keep = "natural_log_exp_and_others"
        assert keep in tables
        return {
            name: (fns if name == keep else set()) for name, fns in tables.items()
        }

    bacc.get_activation_tables = patched
    bacc._act_tables_patched = True


def _build():
    from contextlib import ExitStack

    import concourse.bacc as bacc
    import concourse.tile as tile
    from concourse import mybir

    _patch_act_tables()

    f32 = mybir.dt.float32
    bf16 = mybir.dt.bfloat16
    f8 = mybir.dt.float8e4
    nc = bacc.Bacc("TRN2", target_bir_lowering=False, debug=False, num_devices=N_CORES)
    aps = (
        nc.dram_tensor("x", [C, S], bf16, kind="ExternalInput").ap(),
        nc.dram_tensor("wqkT", [C, 2 * HID], bf16, kind="ExternalInput").ap(),
        nc.dram_tensor("wvT", [C, HID], bf16, kind="ExternalInput").ap(),
        nc.dram_tensor("woT", [HID, C], bf16, kind="ExternalInput").ap(),
        nc.dram_tensor("bout", [128, 2], f32, kind="ExternalInput").ap(),
        nc.dram_tensor("cosT", [128, S], bf16, kind="ExternalInput").ap(),
        nc.dram_tensor("sinT", [128, S], bf16, kind="ExternalInput").ap(),
        nc.dram_tensor("out", [C, S], f32, kind="ExternalOutput").ap(),
    )
    with tile.TileContext(nc) as tc:
        with ExitStack() as ctx:
            _emit(ctx, tc, aps)
    nc.compile()
    return nc


def _get_nc():
    if "nc" not in _CACHE:
        _CACHE["nc"] = _build()
    return _CACHE["nc"]


def _make_in_maps(x, w_qkv, w_out, b_out):
    import ml_dtypes

    bf = ml_dtypes.bfloat16
    xf32 = np.asarray(x, np.float32).reshape(N_CORES, C, S)
    xf = np.ascontiguousarray(xf32).astype(bf)
    wq = np.asarray(w_qkv, np.float32)
    wqkT = np.ascontiguousarray(wq[0 : 2 * HID, :].T).astype(bf)
    wvT = np.ascontiguousarray(wq[2 * HID : 3 * HID, :].T).astype(bf)
    woT = np.ascontiguousarray(np.asarray(w_out, np.float32).T).astype(bf)
    bo = np.ascontiguousarray(np.asarray(b_out, np.float32).reshape(2, 128).T)
    cosT, sinT = _rope_tables()
    shared = {
        "wqkT": wqkT,
        "wvT": wvT,
        "woT": woT,
        "bout": bo,
        "cosT": cosT.astype(bf),
        "sinT": sinT.astype(bf),
    }
    return [dict(shared, x=np.ascontiguousarray(xf[i])) for i in range(N_CORES)]


def _postprocess(res):
    out = np.stack([r["out"] for r in res.results], axis=0)
    return out.reshape(N_CORES, C, 32, 32).astype(np.float32)


def _run(x, w_qkv, w_out, b_out, trace=False):
    from concourse.bass_utils import run_bass_kernel_spmd

    nc = _get_nc()
    in_maps = _make_in_maps(x, w_qkv, w_out, b_out)
    res = run_bass_kernel_spmd(nc, in_maps, core_ids=list(range(N_CORES)), trace=trace)
    return _postprocess(res), res


def kernel(x, w_qkv, w_out, b_out):
    return _run(x, w_qkv, w_out, b_out, trace=False)[0]


# revision 49
# speedup vs baseline: 1.0034x; 1.0034x over previous
"""Trainium2 Bass kernel for AttentionWithRotaryPosEmb (8 cores, data-parallel).

Strategy
--------
Data-parallel over batch: each of the 8 NeuronCores computes one batch element
end-to-end. No collectives needed.

Per-core pipeline (batch x_b is [C=256, S=1024]):
  1. QKV projection: q,k as [o, s] (o = 64h+d), vT as [s, hid] straight out of
     the matmuls -- no transposes needed on-chip. PSUM evacuated by ACT copy
     (f32 -> bf16) so DVE stays free and all later elementwise runs in bf16
     DVE perf modes (2x/4x).
  2. RoPE on q,k in [d, s] layout: rotate-half is a single DVE stream_shuffle
     (32-partition-group permutation), then sin/cos multiplies + add against
     host-precomputed row tables (sign of sin encodes rotate-half; rows with
     d >= 32 have sin=0, cos=1).
  3. L2 norm over the sequence axis: fused square+reduce on DVE
     (tensor_tensor_reduce), rsqrt via exp(-0.5*ln(x)) on ACT (same activation
     table set as the softmax exp -> no table reloads). Both q and k scales
     fold into q.
  4. Attention with transposed softmax: simT[j, i] via matmul(lhsT=k_h,
     rhs=q_h); exp on ACT with scale=10 fused; softmax denominators come free
     from a ones-column appended to vT (65th lhsT column); outT[d, i]
     accumulates over j-blocks in PSUM.
  5. Softmax normalization: the denominator row is evacuated and reshaped via
     DMA to a [16, 64] layout so the DVE reciprocal runs multi-partition
     (0.5us instead of 6.5us single-lane), DMA'd back to a row, broadcast
     across partitions on the (otherwise idle) GPSIMD engine, then one DVE
     multiply. All shuttle DMAs issue from the GPSIMD queue, keeping the Sync
     engine out of the critical path.
  6. Output projection with the bias applied by ACT (per-partition bias AP).
"""

import os
import sys

import numpy as np

if "/opt/trn_rl_repo" not in sys.path:
    sys.path.insert(0, "/opt/trn_rl_repo")

# bisect fallbacks (baseline-proven implementations).  tensor_tensor_reduce
# crashes at runtime on this hw/runtime combo -> always use mul+reduce_sum.
NO_SHUFFLE = os.environ.get("K_NO_SHUFFLE") == "1"
NO_TTR = True
NO_IDBIAS = os.environ.get("K_NO_IDBIAS") == "1"
NO_BCASTMM = os.environ.get("K_NO_BCASTMM") == "1"

HEADS = 8
DH = 64
S = 1024
C = 256
HID = 512
ROT = 32
HALF = 16
SCALE = 10.0
N_CORES = 8

# rotate-half as a 32-partition-group shuffle: swap the 16-row halves
SHUF_MASK = list(range(16, 32)) + list(range(16))

_CACHE = {}


def _rope_tables():
    """Row-patterned cos/sin tables [128, S] matching the q/k SBUF layout.

    Partition row r holds o-row (128t + r) of q/k tile t, i.e. head 2t + (r>=64)
    and d = r % 64.  Rows d in [0,16) get cos(i*invf[d]) / -sin(i*invf[d]);
    rows d in [16,32) get cos(i*invf[d-16]) / +sin(i*invf[d-16]); rows d >= 32
    get cos=1, sin=0 (identity).  The sign of sin encodes rotate_half.
    """
    inv = (
        1.0 / (np.float32(10000.0) ** (np.arange(0, ROT, 2, dtype=np.float32) / np.float32(ROT)))
    ).astype(np.float32)
    ang = (np.arange(S, dtype=np.float32)[None, :] * inv[:, None]).astype(np.float32)
    cos16 = np.cos(ang).astype(np.float32)  # [16, S]
    sin16 = np.sin(ang).astype(np.float32)
    cosT = np.ones((128, S), np.float32)
    sinT = np.zeros((128, S), np.float32)
    for r in range(128):
        d = r % 64
        if d < HALF:
            cosT[r] = cos16[d]
            sinT[r] = -sin16[d]
        elif d < ROT:
            cosT[r] = cos16[d - HALF]
            sinT[r] = sin16[d - HALF]
    return cosT, sinT


def _emit(ctx, tc, aps):
    import concourse.bass as bass  # noqa: F401
    from concourse import mybir

    f32 = mybir.dt.float32
    bf16 = mybir.dt.bfloat16
    f8 = mybir.dt.float8e4
    DR = mybir.MatmulPerfMode.DoubleRow
    AF = mybir.ActivationFunctionType
    ALU = mybir.AluOpType
    nc = tc.nc
    x_d, wqk_d, wv_d, wo_d, bo_d, cos_d, sin_d, out_d = aps

    singles = ctx.enter_context(tc.tile_pool(name="singles", bufs=1))
    wk = ctx.enter_context(tc.tile_pool(name="wk", bufs=3))
    ppm = ctx.enter_context(tc.tile_pool(name="ppm", bufs=2, space="PSUM"))
    ppo = ctx.enter_context(tc.tile_pool(name="ppo", bufs=2, space="PSUM"))

    # ---- persistent SBUF ----
    sb_x = singles.tile([128, 2, S], bf16)
    sb_wqk = singles.tile([128, 2, 2 * HID], bf16)
    sb_wv = singles.tile([128, 2, HID], bf16)
    sb_wo = singles.tile([128, 4, C], bf16)
    sb_bo = singles.tile([128, 2], f32)
    sb_cos = singles.tile([128, S], bf16)
    sb_sin = singles.tile([128, S], bf16)
    sb_q = singles.tile([128, 4, S], bf16)   # raw q -> roped q (pre-scale)
    sb_k = singles.tile([128, 4, S], bf16)   # raw k (k*cos staging in-place)
    sb_qb = singles.tile([128, 4, S], bf16)  # normalized q, sim operand
    sb_kb = singles.tile([128, 4, S], bf16)  # roped k, sim operand
    sb_vx = singles.tile([128, 8, HEADS * (DH + 1)], bf16)
    sb_or = singles.tile([128, 4, S], bf16)  # attention out, unnormalized
    sb_o = singles.tile([128, 4, S], bf16)   # attention out, proj operand
    sb_stat = singles.tile([128, 16], f32)
    den_sh = singles.tile([128, 8, 8], f32)     # partition-spread denominators
    rcp_sh = singles.tile([128, 8, 8], bf16)    # reciprocals (bf16: bcast operand)
    ones_col = singles.tile([1, 64], bf16)      # lhsT for the rcp row broadcast
    acc = singles.tile([128, 2, S], f32)        # output-projection accumulator

    # ---- input loads (c-block granularity so QKV matmuls can start early) ----
    nc.sync.dma_start(out=sb_wqk[:, 0, :], in_=wqk_d[0:128, :])
    nc.sync.dma_start(out=sb_x[:, 0, :], in_=x_d[0:128, :])
    nc.sync.dma_start(out=sb_wqk[:, 1, :], in_=wqk_d[128:256, :])
    nc.sync.dma_start(out=sb_x[:, 1, :], in_=x_d[128:256, :])
    nc.sync.dma_start(out=sb_wv[:, 0, :], in_=wv_d[0:128, :])
    nc.sync.dma_start(out=sb_wv[:, 1, :], in_=wv_d[128:256, :])
    nc.sync.dma_start(out=sb_cos[:, :], in_=cos_d[:, :])
    nc.sync.dma_start(out=sb_sin[:, :], in_=sin_d[:, :])
    for kk in range(4):
        nc.sync.dma_start(out=sb_wo[:, kk, :], in_=wo_d[kk * 128 : (kk + 1) * 128, :])
    nc.sync.dma_start(out=sb_bo[:, :], in_=bo_d[:, :])

    # ones column per head in vT_ext -> free softmax denominators
    vx4 = sb_vx.rearrange("p j (h e) -> p j h e", e=DH + 1)
    nc.vector.memset(vx4[:, :, :, DH : DH + 1], 1.0)
    nc.vector.memset(ones_col[:, :], 1.0)

    def qk_block(ob):
        """q (ob<4) / k (ob>=4) projection o-block -> SBUF bf16 via ACT."""
        ps = ppm.tile([128, S], f32, tag="mm", name=f"ps_qk{ob}")
        for nn in range(2):
            for kk in range(2):
                nc.tensor.matmul(
                    ps[:, nn * 512 : (nn + 1) * 512],
                    lhsT=(sb_wqk[:, kk, ob * 128 : (ob + 1) * 128]),
                    rhs=(sb_x[:, kk, nn * 512 : (nn + 1) * 512]),
                    start=(kk == 0),
                    stop=(kk == 1),
                )
        dst3 = sb_q if ob < 4 else sb_k
        nc.scalar.copy(out=dst3[:, ob % 4, :], in_=ps[:, :])

    def rope_norm(t):
        """RoPE + l2-norm stats for q/k tile t; fold both scales into q."""
        for src3, col in ((sb_q, 0), (sb_k, 4)):
            cur = src3[:, t, :]
            rot = wk.tile([128, S], bf16, tag="rot", name=f"rot{t}_{col}", bufs=2)
            # rotate-half: swap 16-row halves of each 32-partition group
            if NO_SHUFFLE:
                for base in range(0, 128, 32):
                    nc.sync.dma_start(
                        out=rot[base : base + 16, :], in_=src3[base + 16 : base + 32, t, :]
                    )
                    nc.sync.dma_start(
                        out=rot[base + 16 : base + 32, :], in_=src3[base : base + 16, t, :]
                    )
            else:
                nc.vector.stream_shuffle(out=rot[:, :], in_=cur, mask=SHUF_MASK)
            nc.vector.tensor_mul(out=rot[:, :], in0=rot[:, :], in1=sb_sin[:, :])
            nc.vector.tensor_mul(out=cur, in0=cur, in1=sb_cos[:, :])
            adddst = cur if col == 0 else sb_kb[:, t, :]
            nc.vector.tensor_add(out=adddst, in0=cur, in1=rot[:, :])
            # fused square + sum over s -> stat col
            sq = wk.tile([128, S], bf16, tag="sq", name=f"sq{t}_{col}", bufs=2)
            if NO_TTR:
                nc.vector.tensor_mul(out=sq[:, :], in0=adddst, in1=adddst)
                nc.vector.reduce_sum(
                    out=sb_stat[:, col + t : col + t + 1], in_=sq[:, :],
                    axis=mybir.AxisListType.X,
                )
            else:
                nc.vector.tensor_tensor_reduce(
                    out=sq[:, :], in0=adddst, in1=adddst, scale=1.0, scalar=0.0,
                    op0=ALU.mult, op1=ALU.add,
                    accum_out=sb_stat[:, col + t : col + t + 1],
                )
        # rs_comb = (ssq_q * ssq_k)^-1/2 = exp(-0.5*(ln q + ln k)); the x256
        # from the fp8 qk weights cancels (sim is scale-invariant through it)
        nc.scalar.activation(
            out=sb_stat[:, 8 + t : 9 + t], in_=sb_stat[:, t : t + 1],
            func=AF.Ln, bias=0.0,
        )
        nc.scalar.activation(
            out=sb_stat[:, 12 + t : 13 + t], in_=sb_stat[:, 4 + t : 5 + t],
            func=AF.Ln, bias=0.0,
        )
        nc.vector.tensor_add(
            out=sb_stat[:, 8 + t : 9 + t],
            in0=sb_stat[:, 8 + t : 9 + t],
            in1=sb_stat[:, 12 + t : 13 + t],
        )
        nc.scalar.activation(
            out=sb_stat[:, 12 + t : 13 + t], in_=sb_stat[:, 8 + t : 9 + t],
            func=AF.Exp, scale=-0.5,
        )
        nc.vector.tensor_scalar_mul(
            out=sb_qb[:, t, :], in0=sb_q[:, t, :],
            scalar1=sb_stat[:, 12 + t : 13 + t],
        )

    def v_block(jb):
        """vT s-block: [s_jb, hid] straight from matmul, strided into vx ext."""
        psv = ppo.tile([128, 512], f32, tag="ov", name=f"psv{jb}")
        for kk in range(2):
            nc.tensor.matmul(
                psv[:, :],
                lhsT=(sb_x[:, kk, jb * 128 : (jb + 1) * 128]),
                rhs=(sb_wv[:, kk, :]),
                start=(kk == 0),
                stop=(kk == 1),
            )
        dst = sb_vx[:, jb, :].rearrange("p (h e) -> p h e", e=DH + 1)[:, :, 0:DH]
        src = psv.rearrange("p (h d) -> p h d", d=DH)
        nc.vector.tensor_copy(out=dst, in_=src)  # f32 PSUM -> bf16 SBUF cast

    def attn_head(h):
        t, half = h // 2, h % 2
        b0 = 64 * half
        q_h = sb_qb[b0 : b0 + 64, t, :]
        k_h = sb_kb[b0 : b0 + 64, t, :]
        pso = ppo.tile([65, S], f32, tag="ov", name=f"pso{h}")
        for jb in range(8):
            pss = ppm.tile([128, S], f32, tag="mm", name=f"pss{h}_{jb}")
            for nn in range(2):
                nc.tensor.matmul(
                    pss[:, nn * 512 : (nn + 1) * 512],
                    lhsT=(k_h[:, jb * 128 : (jb + 1) * 128]),
                    rhs=(q_h[:, nn * 512 : (nn + 1) * 512]),
                    start=True,
                    stop=True,
                )
            et = wk.tile([128, S], bf16, tag="et", name=f"et{h}_{jb}", bufs=3)
            nc.scalar.activation(out=et[:, :], in_=pss[:, :], func=AF.Exp, scale=SCALE)
            for nn in range(2):
                nc.tensor.matmul(
                    pso[0:65, nn * 512 : (nn + 1) * 512],
                    lhsT=(sb_vx[:, jb, h * (DH + 1) : (h + 1) * (DH + 1)]),
                    rhs=(et[:, nn * 512 : (nn + 1) * 512]),
                    start=(jb == 0),
                    stop=(jb == 7),
                )
        # park the unnormalized output + denominator row; frees the PSUM buf
        # without waiting for the reciprocal roundtrip
        nc.vector.tensor_copy(out=sb_or[b0 : b0 + 64, t, :], in_=pso[0:64, :])
        dstage = wk.tile([1, S], f32, tag="dstage", name=f"dstage{h}", bufs=2)
        nc.vector.tensor_copy(out=dstage[:, :], in_=pso[64:65, :])
        nc.sync.dma_start(out=den_sh[:, h, :], in_=dstage[:, :])
        with nc.allow_low_precision("softmax denominator reciprocal in bf16"):
            nc.vector.reciprocal(out=rcp_sh[:, h, :], in_=den_sh[:, h, :])

    def attn_norm(h):
        """Broadcast 1/den across partitions via a K=1 PE matmul, then scale.

        Emitted one head late so the PE queue never stalls on the reciprocal
        roundtrip; the broadcast tile borrows a ppm slot (PSUM is full during
        attention).
        """
        t, half = h // 2, h % 2
        b0 = 64 * half
        if NO_BCASTMM:
            # both-SBUF tensor_tensor needs equal base partitions: use a full
            # [128, S] tile and fill rows b0..b0+64 via the doubling chain
            rbc = wk.tile([128, S], bf16, tag="rbc", name=f"rbc{h}", bufs=2)
            nc.sync.dma_start(out=rbc[b0 : b0 + 1, :], in_=rcp_sh[:, h, :])
            w = 1
            while w < 64:
                nc.sync.dma_start(
                    out=rbc[b0 + w : b0 + 2 * w, :], in_=rbc[b0 : b0 + w, :]
                )
                w *= 2
            nc.vector.tensor_mul(
                out=sb_o[b0 : b0 + 64, t, :],
                in0=sb_or[b0 : b0 + 64, t, :],
                in1=rbc[b0 : b0 + 64, :],
            )
            return
        rrow = wk.tile([1, S], bf16, tag="rrow", name=f"rrow{h}", bufs=2)
        nc.sync.dma_start(out=rrow[:, :], in_=rcp_sh[:, h, :])
        pbc = ppm.tile([64, S], f32, tag="mm", name=f"pbc{h}")
        for nn in range(2):
            nc.tensor.matmul(
                pbc[:, nn * 512 : (nn + 1) * 512],
                lhsT=ones_col[0:1, :],
                rhs=rrow[0:1, nn * 512 : (nn + 1) * 512],
                start=True,
                stop=True,
            )
        nc.vector.tensor_mul(
            out=sb_o[b0 : b0 + 64, t, :],
            in0=sb_or[b0 : b0 + 64, t, :],
            in1=pbc[:, :],
        )

    def proj_t(t):
        """Output-projection partial for attention tile t, accumulated into
        SBUF f32 so the matmuls overlap attention instead of serializing at
        the tail.  t=0 evacuates via ACT with the bias fused; later t's add
        the PSUM partial on DVE; t=3 streams the result out."""
        for ob in range(2):
            psf = ppm.tile([128, S], f32, tag="mm", name=f"psf{ob}_{t}")
            for nn in range(2):
                nc.tensor.matmul(
                    psf[:, nn * 512 : (nn + 1) * 512],
                    lhsT=(sb_wo[:, t, ob * 128 : (ob + 1) * 128]),
                    rhs=(sb_o[:, t, nn * 512 : (nn + 1) * 512]),
                    start=True,
                    stop=True,
                )
            if t == 0:
                nc.scalar.activation(
                    out=acc[:, ob, :], in_=psf[:, :], func=AF.Identity,
                    bias=sb_bo[:, ob : ob + 1],
                )
            else:
                nc.vector.tensor_add(
                    out=acc[:, ob, :], in0=psf[:, :], in1=acc[:, ob, :]
                )
                if t == 3:
                    nc.sync.dma_start(
                        out=out_d[ob * 128 : (ob + 1) * 128, :], in_=acc[:, ob, :]
                    )

    # ---- emission order == scheduling priority ----
    qk_block(0)
    qk_block(4)
    rope_norm(0)
    for jb in range(8):
        v_block(jb)
    qk_block(1)
    qk_block(5)
    rope_norm(1)
    attn_head(0)
    attn_head(1)
    attn_norm(0)
    qk_block(2)
    qk_block(6)
    rope_norm(2)
    attn_head(2)
    attn_norm(1)
    attn_head(3)
    attn_norm(2)
    qk_block(3)
    qk_block(7)
    rope_norm(3)
    attn_head(4)
    attn_norm(3)
    attn_head(5)
    attn_norm(4)
    proj_t(0)
    attn_head(6)
    attn_norm(5)
    proj_t(1)
    attn_head(7)
    attn_norm(6)
    proj_t(2)
    attn_norm(7)
    proj_t(3)


def _patch_act_tables():
    """Steer the act-table-load pass to one set covering Exp+Ln+Copy.

    The default pass picks the first table set containing each activation
    function, which ping-pongs between exp_and_others and natural_log
    (~2.7us per reload).  Emptying every other set forces all activations
    onto natural_log_exp_and_others -> exactly one load.
    """
    import concourse.bacc as bacc

    if getattr(bacc, "_act_tables_patched", False):
        return
    import concourse.hw_specs as hw_specs

    orig = hw_specs.get_activation_tables

    def patched(arch):
        tables = orig(arch)
        keep = "natural_log_exp_and_others"
        assert keep in tables
        return {
            name: (fns if name == keep else set()) for name, fns in tables.items()
        }

    bacc.get_activation_tables = patched
    bacc._act_tables_patched = True


def _build():
    from contextlib import ExitStack

    import concourse.bacc as bacc
    import concourse.tile as tile
    from concourse import mybir

    _patch_act_tables()

    f32 = mybir.dt.float32
    bf16 = mybir.dt.bfloat16
    f8 = mybir.dt.float8e4
    nc = bacc.Bacc("TRN2", target_bir_lowering=False, debug=False, num_devices=N_CORES)
    aps = (
        nc.dram_tensor("x", [C, S], bf16, kind="ExternalInput").ap(),
        nc.dram_tensor("wqkT", [C, 2 * HID], bf16, kind="ExternalInput").ap(),
        nc.dram_tensor("wvT", [C, HID], bf16, kind="ExternalInput").ap(),
        nc.dram_tensor("woT", [HID, C], bf16, kind="ExternalInput").ap(),
        nc.dram_tensor("bout", [128, 2], f32, kind="ExternalInput").ap(),
        nc.dram_tensor("cosT", [128, S], bf16, kind="ExternalInput").ap(),
        nc.dram_tensor("sinT", [128, S], bf16, kind="ExternalInput").ap(),
        nc.dram_tensor("out", [C, S], f32, kind="ExternalOutput").ap(),
    )
    with tile.TileContext(nc) as tc:
        with ExitStack() as ctx:
            _emit(ctx, tc, aps)
    nc.compile()
    return nc


def _get_nc():
    if "nc" not in _CACHE:
        _CACHE["nc"] = _build()
    return _CACHE["nc"]


def _make_in_maps(x, w_qkv, w_out, b_out):
    import ml_dtypes

    bf = ml_dtypes.bfloat16
    xf32 = np.asarray(x, np.float32).reshape(N_CORES, C, S)
    xf = np.ascontiguousarray(xf32).astype(bf)
    wq = np.asarray(w_qkv, np.float32)
    wqkT = np.ascontiguousarray(wq[0 : 2 * HID, :].T).astype(bf)
    wvT = np.ascontiguousarray(wq[2 * HID : 3 * HID, :].T).astype(bf)
    woT = np.ascontiguousarray(np.asarray(w_out, np.float32).T).astype(bf)
    bo = np.ascontiguousarray(np.asarray(b_out, np.float32).reshape(2, 128).T)
    cosT, sinT = _rope_tables()
    shared = {
        "wqkT": wqkT,
        "wvT": wvT,
        "woT": woT,
        "bout": bo,
        "cosT": cosT.astype(bf),
        "sinT": sinT.astype(bf),
    }
    return [dict(shared, x=np.ascontiguousarray(xf[i])) for i in range(N_CORES)]


def _postprocess(res):
    out = np.stack([r["out"] for r in res.results], axis=0)
    return out.reshape(N_CORES, C, 32, 32).astype(np.float32)


def _run(x, w_qkv, w_out, b_out, trace=False):
    from concourse.bass_utils import run_bass_kernel_spmd

    nc = _get_nc()
    in_maps = _make_in_maps(x, w_qkv, w_out, b_out)
    res = run_bass_kernel_spmd(nc, in_maps, core_ids=list(range(N_CORES)), trace=trace)
    return _postprocess(res), res


def kernel(x, w_qkv, w_out, b_out):
    return _run(x, w_qkv, w_out, b_out, trace=False)[0]


# revision 68
# speedup vs baseline: 1.5363x; 1.5311x over previous
"""Trainium2 Bass kernel for AttentionWithRotaryPosEmb (8 cores, data-parallel).

Strategy
--------
Data-parallel over batch: each of the 8 NeuronCores computes one batch element
end-to-end. No collectives needed.

Per-core pipeline (batch x_b is [C=256, S=1024]):
  1. QKV projection: q,k as [o, s] (o = 64h+d), vT as [s, hid] straight out of
     the matmuls -- no transposes needed on-chip. PSUM evacuated by ACT copy
     (f32 -> bf16) so DVE stays free and all later elementwise runs in bf16
     DVE perf modes (2x/4x).
  2. RoPE on q,k in [d, s] layout: rotate-half is a single DVE stream_shuffle
     (32-partition-group permutation), then sin/cos multiplies + add against
     host-precomputed row tables (sign of sin encodes rotate-half; rows with
     d >= 32 have sin=0, cos=1).
  3. L2 norm over the sequence axis: fused square+reduce on DVE
     (tensor_tensor_reduce), rsqrt via exp(-0.5*ln(x)) on ACT (same activation
     table set as the softmax exp -> no table reloads). Both q and k scales
     fold into q.
  4. Attention with transposed softmax: simT[j, i] via matmul(lhsT=k_h,
     rhs=q_h); exp on ACT with scale=10 fused; softmax denominators come free
     from a ones-column appended to vT (65th lhsT column); outT[d, i]
     accumulates over j-blocks in PSUM.
  5. Softmax normalization: the denominator row is evacuated and reshaped via
     DMA to a [16, 64] layout so the DVE reciprocal runs multi-partition
     (0.5us instead of 6.5us single-lane), DMA'd back to a row, broadcast
     across partitions on the (otherwise idle) GPSIMD engine, then one DVE
     multiply. All shuttle DMAs issue from the GPSIMD queue, keeping the Sync
     engine out of the critical path.
  6. Output projection with the bias applied by ACT (per-partition bias AP).
"""

import os
import sys

import numpy as np

if "/opt/trn_rl_repo" not in sys.path:
    sys.path.insert(0, "/opt/trn_rl_repo")

# bisect fallbacks (baseline-proven implementations).  tensor_tensor_reduce
# crashes at runtime on this hw/runtime combo -> always use mul+reduce_sum.
NO_SHUFFLE = os.environ.get("K_NO_SHUFFLE") == "1"
NO_TTR = True
NO_IDBIAS = os.environ.get("K_NO_IDBIAS") == "1"
NO_BCASTMM = os.environ.get("K_NO_BCASTMM") == "1"
TAILPROJ = os.environ.get("K_TAILPROJ", "1") == "1"
FP8 = os.environ.get("K_FP8") == "1"

HEADS = 8
DH = 64
S = 1024
C = 256
HID = 512
ROT = 32
HALF = 16
SCALE = 10.0
N_CORES = 8

# rotate-half as a 32-partition-group shuffle: swap the 16-row halves
SHUF_MASK = list(range(16, 32)) + list(range(16))

_CACHE = {}


def _rope_tables():
    """Row-patterned cos/sin tables [128, S] matching the q/k SBUF layout.

    Partition row r holds o-row (128t + r) of q/k tile t, i.e. head 2t + (r>=64)
    and d = r % 64.  Rows d in [0,16) get cos(i*invf[d]) / -sin(i*invf[d]);
    rows d in [16,32) get cos(i*invf[d-16]) / +sin(i*invf[d-16]); rows d >= 32
    get cos=1, sin=0 (identity).  The sign of sin encodes rotate_half.
    """
    inv = (
        1.0 / (np.float32(10000.0) ** (np.arange(0, ROT, 2, dtype=np.float32) / np.float32(ROT)))
    ).astype(np.float32)
    ang = (np.arange(S, dtype=np.float32)[None, :] * inv[:, None]).astype(np.float32)
    cos16 = np.cos(ang).astype(np.float32)  # [16, S]
    sin16 = np.sin(ang).astype(np.float32)
    cosT = np.ones((128, S), np.float32)
    sinT = np.zeros((128, S), np.float32)
    for r in range(128):
        d = r % 64
        if d < HALF:
            cosT[r] = cos16[d]
            sinT[r] = -sin16[d]
        elif d < ROT:
            cosT[r] = cos16[d - HALF]
            sinT[r] = sin16[d - HALF]
    return cosT, sinT


def _emit(ctx, tc, aps):
    import concourse.bass as bass  # noqa: F401
    from concourse import mybir

    f32 = mybir.dt.float32
    bf16 = mybir.dt.bfloat16
    f8 = mybir.dt.float8e4
    DR = mybir.MatmulPerfMode.DoubleRow
    AF = mybir.ActivationFunctionType
    ALU = mybir.AluOpType
    nc = tc.nc
    x_d, wqk_d, x8_d, wqk8_d, wv_d, wo_d, bo_d, cos_d, sin_d, out_d = aps

    singles = ctx.enter_context(tc.tile_pool(name="singles", bufs=1))
    wk = ctx.enter_context(tc.tile_pool(name="wk", bufs=3))
    ppm = ctx.enter_context(tc.tile_pool(name="ppm", bufs=2, space="PSUM"))
    ppo = ctx.enter_context(tc.tile_pool(name="ppo", bufs=2, space="PSUM"))

    # ---- persistent SBUF ----
    sb_x = singles.tile([128, 2, S], bf16)
    sb_wv = singles.tile([128, 2, HID], bf16)
    if FP8:
        sb_x8 = singles.tile([128, 2, S], f8)
        sb_wqk8 = singles.tile([128, 2, 2 * HID], f8)
        q8a = singles.tile([128, 4, S], f8)
        k8a = singles.tile([128, 4, S], f8)
        q8 = singles.tile([64, 4, 2, S], f8)
        k8 = singles.tile([64, 4, 2, S], f8)
        ln256 = singles.tile([128, 1], f32)
    else:
        sb_wqk = singles.tile([128, 2, 2 * HID], bf16)
    sb_wo = singles.tile([128, 4, C], bf16)
    sb_bo = singles.tile([128, 2], f32)
    sb_cos = singles.tile([128, S], bf16)
    sb_sin = singles.tile([128, S], bf16)
    sb_q = singles.tile([128, 4, S], bf16)   # raw q -> roped q (pre-scale)
    sb_k = singles.tile([128, 4, S], bf16)   # raw k (k*cos staging in-place)
    sb_qb = singles.tile([128, 4, S], bf16)  # normalized q, sim operand
    sb_kb = singles.tile([128, 4, S], bf16)  # roped k, sim operand
    sb_vx = singles.tile([128, 8, HEADS * (DH + 1)], bf16)
    sb_or = singles.tile([128, 4, S], bf16)  # attention out, unnormalized
    sb_o = singles.tile([128, 4, S], bf16)   # attention out, proj operand
    sb_stat = singles.tile([128, 16], f32)
    den_sh = singles.tile([128, 8, 8], f32)     # partition-spread denominators
    rcp_sh = singles.tile([128, 8, 8], bf16)    # reciprocals (bf16: bcast operand)
    ones_col = singles.tile([1, 64], bf16)      # lhsT for the rcp row broadcast
    acc = singles.tile([128, 2, S], f32)        # output-projection accumulator

    # ---- input loads (c-block granularity so QKV matmuls can start early) ----
    if FP8:
        nc.sync.dma_start(out=sb_wqk8[:, :, :], in_=wqk8_d[:, :])
        nc.sync.dma_start(out=sb_x8[:, :, :], in_=x8_d[:, :])
    else:
        nc.sync.dma_start(out=sb_wqk[:, 0, :], in_=wqk_d[0:128, :])
        nc.sync.dma_start(out=sb_wqk[:, 1, :], in_=wqk_d[128:256, :])
    nc.sync.dma_start(out=sb_x[:, 0, :], in_=x_d[0:128, :])
    nc.sync.dma_start(out=sb_x[:, 1, :], in_=x_d[128:256, :])
    nc.sync.dma_start(out=sb_wv[:, 0, :], in_=wv_d[0:128, :])
    nc.sync.dma_start(out=sb_wv[:, 1, :], in_=wv_d[128:256, :])
    nc.sync.dma_start(out=sb_cos[:, :], in_=cos_d[:, :])
    nc.sync.dma_start(out=sb_sin[:, :], in_=sin_d[:, :])
    for kk in range(4):
        nc.sync.dma_start(out=sb_wo[:, kk, :], in_=wo_d[kk * 128 : (kk + 1) * 128, :])
    nc.sync.dma_start(out=sb_bo[:, :], in_=bo_d[:, :])

    # ones column per head in vT_ext -> free softmax denominators
    vx4 = sb_vx.rearrange("p j (h e) -> p j h e", e=DH + 1)
    nc.vector.memset(vx4[:, :, :, DH : DH + 1], 1.0)
    nc.vector.memset(ones_col[:, :], 1.0)
    if FP8:
        nc.vector.memset(ln256[:, :], float(np.log(256.0)))

    def qk_block(ob):
        """q (ob<4) / k (ob>=4) projection o-block -> SBUF bf16 via ACT."""
        ps = ppm.tile([128, S], f32, tag="mm", name=f"ps_qk{ob}")
        for nn in range(2):
            if FP8:
                nc.tensor.matmul(
                    ps[:, nn * 512 : (nn + 1) * 512],
                    lhsT=(sb_wqk8[:, :, ob * 128 : (ob + 1) * 128]),
                    rhs=(sb_x8[:, :, nn * 512 : (nn + 1) * 512]),
                    start=True,
                    stop=True,
                    perf_mode=DR,
                )
                continue
            for kk in range(2):
                nc.tensor.matmul(
                    ps[:, nn * 512 : (nn + 1) * 512],
                    lhsT=(sb_wqk[:, kk, ob * 128 : (ob + 1) * 128]),
                    rhs=(sb_x[:, kk, nn * 512 : (nn + 1) * 512]),
                    start=(kk == 0),
                    stop=(kk == 1),
                )
        dst3 = sb_q if ob < 4 else sb_k
        nc.scalar.copy(out=dst3[:, ob % 4, :], in_=ps[:, :])

    def rope_norm(t):
        """RoPE + l2-norm stats for q/k tile t; fold both scales into q."""
        for src3, col in ((sb_q, 0), (sb_k, 4)):
            cur = src3[:, t, :]
            rot = wk.tile([128, S], bf16, tag="rot", name=f"rot{t}_{col}", bufs=2)
            # rotate-half: swap 16-row halves of each 32-partition group
            if NO_SHUFFLE:
                for base in range(0, 128, 32):
                    nc.sync.dma_start(
                        out=rot[base : base + 16, :], in_=src3[base + 16 : base + 32, t, :]
                    )
                    nc.sync.dma_start(
                        out=rot[base + 16 : base + 32, :], in_=src3[base : base + 16, t, :]
                    )
            else:
                nc.vector.stream_shuffle(out=rot[:, :], in_=cur, mask=SHUF_MASK)
            nc.vector.tensor_mul(out=rot[:, :], in0=rot[:, :], in1=sb_sin[:, :])
            nc.vector.tensor_mul(out=cur, in0=cur, in1=sb_cos[:, :])
            adddst = cur if col == 0 else sb_kb[:, t, :]
            nc.vector.tensor_add(out=adddst, in0=cur, in1=rot[:, :])
            # fused square + sum over s -> stat col.  Tile 0 is the attention
            # warm-up critical path: run it on ACT (idle then) to shorten the
            # DVE chain; ACT is exp-saturated for later tiles.
            sq = wk.tile([128, S], bf16, tag="sq", name=f"sq{t}_{col}", bufs=2)
            if t == 0:
                nc.scalar.activation(
                    out=sq[:, :], in_=adddst, func=AF.Square,
                    accum_out=sb_stat[:, col + t : col + t + 1],
                )
            elif NO_TTR:
                nc.vector.tensor_mul(out=sq[:, :], in0=adddst, in1=adddst)
                nc.vector.reduce_sum(
                    out=sb_stat[:, col + t : col + t + 1], in_=sq[:, :],
                    axis=mybir.AxisListType.X,
                )
            else:
                nc.vector.tensor_tensor_reduce(
                    out=sq[:, :], in0=adddst, in1=adddst, scale=1.0, scalar=0.0,
                    op0=ALU.mult, op1=ALU.add,
                    accum_out=sb_stat[:, col + t : col + t + 1],
                )
        # rs_comb = (ssq_q * ssq_k)^-1/2 = exp(-0.5*(ln q + ln k)); the x256
        # from the fp8 qk weights cancels (sim is scale-invariant through it)
        nc.scalar.activation(
            out=sb_stat[:, 8 + t : 9 + t], in_=sb_stat[:, t : t + 1],
            func=AF.Ln, bias=0.0,
        )
        nc.scalar.activation(
            out=sb_stat[:, 12 + t : 13 + t], in_=sb_stat[:, 4 + t : 5 + t],
            func=AF.Ln, bias=0.0,
        )
        nc.vector.tensor_add(
            out=sb_stat[:, 8 + t : 9 + t],
            in0=sb_stat[:, 8 + t : 9 + t],
            in1=sb_stat[:, 12 + t : 13 + t],
        )
        if FP8:
            nc.scalar.activation(
                out=sb_stat[:, 12 + t : 13 + t], in_=sb_stat[:, 8 + t : 9 + t],
                func=AF.Exp, scale=-0.25, bias=ln256[:, 0:1],
            )
            sc_col = sb_stat[:, 12 + t : 13 + t]
            nc.vector.tensor_scalar_mul(out=q8a[:, t, :], in0=sb_q[:, t, :], scalar1=sc_col)
            nc.vector.tensor_scalar_mul(out=k8a[:, t, :], in0=sb_kb[:, t, :], scalar1=sc_col)
            for src8, dst8 in ((q8a, q8), (k8a, k8)):
                for hh in range(2):
                    for i in range(2):
                        nc.sync.dma_start(
                            out=dst8[32 * hh : 32 * hh + 32, t, i, :],
                            in_=src8[64 * hh + 32 * i : 64 * hh + 32 * i + 32, t, :],
                        )
        else:
            nc.scalar.activation(
                out=sb_stat[:, 12 + t : 13 + t], in_=sb_stat[:, 8 + t : 9 + t],
                func=AF.Exp, scale=-0.5,
            )
            nc.vector.tensor_scalar_mul(
                out=sb_qb[:, t, :], in0=sb_q[:, t, :],
                scalar1=sb_stat[:, 12 + t : 13 + t],
            )

    def v_block(jb):
        """vT s-block: [s_jb, hid] straight from matmul, strided into vx ext."""
        psv = ppo.tile([128, 512], f32, tag="ov", name=f"psv{jb}")
        for kk in range(2):
            nc.tensor.matmul(
                psv[:, :],
                lhsT=(sb_x[:, kk, jb * 128 : (jb + 1) * 128]),
                rhs=(sb_wv[:, kk, :]),
                start=(kk == 0),
                stop=(kk == 1),
            )
        dst = sb_vx[:, jb, :].rearrange("p (h e) -> p h e", e=DH + 1)[:, :, 0:DH]
        src = psv.rearrange("p (h d) -> p h d", d=DH)
        nc.vector.tensor_copy(out=dst, in_=src)  # f32 PSUM -> bf16 SBUF cast

    def attn_head(h):
        t, half = h // 2, h % 2
        b0 = 64 * half
        b1 = 32 * half
        pso = ppo.tile([65, S], f32, tag="ov", name=f"pso{h}")
        for jb in range(8):
            pss = ppm.tile([128, S], f32, tag="mm", name=f"pss{h}_{jb}")
            for nn in range(2):
                if FP8:
                    nc.tensor.matmul(
                        pss[:, nn * 512 : (nn + 1) * 512],
                        lhsT=(k8[b1 : b1 + 32, t, :, jb * 128 : (jb + 1) * 128]),
                        rhs=(q8[b1 : b1 + 32, t, :, nn * 512 : (nn + 1) * 512]),
                        start=True,
                        stop=True,
                        perf_mode=DR,
                    )
                else:
                    nc.tensor.matmul(
                        pss[:, nn * 512 : (nn + 1) * 512],
                        lhsT=(sb_kb[b0 : b0 + 64, t, jb * 128 : (jb + 1) * 128]),
                        rhs=(sb_qb[b0 : b0 + 64, t, nn * 512 : (nn + 1) * 512]),
                        start=True,
                        stop=True,
                    )
            et = wk.tile([128, S], bf16, tag="et", name=f"et{h}_{jb}", bufs=3)
            nc.scalar.activation(
                out=et[:, :], in_=pss[:, :], func=AF.Exp,
                scale=(SCALE / 65536.0 if FP8 else SCALE),
            )
            for nn in range(2):
                nc.tensor.matmul(
                    pso[0:65, nn * 512 : (nn + 1) * 512],
                    lhsT=(sb_vx[:, jb, h * (DH + 1) : (h + 1) * (DH + 1)]),
                    rhs=(et[:, nn * 512 : (nn + 1) * 512]),
                    start=(jb == 0),
                    stop=(jb == 7),
                )
        # park the unnormalized output + denominator row; frees the PSUM buf
        # without waiting for the reciprocal roundtrip.  For the last head the
        # output parks via ACT (idle once the final exp retires), halving the
        # tail's serial DVE chain.
        if h == HEADS - 1:
            nc.scalar.copy(out=sb_or[b0 : b0 + 64, t, :], in_=pso[0:64, :])
        else:
            nc.vector.tensor_copy(out=sb_or[b0 : b0 + 64, t, :], in_=pso[0:64, :])
        dstage = wk.tile([1, S], f32, tag="dstage", name=f"dstage{h}", bufs=2)
        nc.vector.tensor_copy(out=dstage[:, :], in_=pso[64:65, :])
        nc.sync.dma_start(out=den_sh[:, h, :], in_=dstage[:, :])
        with nc.allow_low_precision("softmax denominator reciprocal in bf16"):
            nc.vector.reciprocal(out=rcp_sh[:, h, :], in_=den_sh[:, h, :])

    def attn_norm(h):
        """Broadcast 1/den across partitions via a K=1 PE matmul, then scale.

        Emitted one head late so the PE queue never stalls on the reciprocal
        roundtrip; the broadcast tile borrows a ppm slot (PSUM is full during
        attention).
        """
        t, half = h // 2, h % 2
        b0 = 64 * half
        if NO_BCASTMM:
            # both-SBUF tensor_tensor needs equal base partitions: use a full
            # [128, S] tile and fill rows b0..b0+64 via the doubling chain
            rbc = wk.tile([128, S], bf16, tag="rbc", name=f"rbc{h}", bufs=2)
            nc.sync.dma_start(out=rbc[b0 : b0 + 1, :], in_=rcp_sh[:, h, :])
            w = 1
            while w < 64:
                nc.sync.dma_start(
                    out=rbc[b0 + w : b0 + 2 * w, :], in_=rbc[b0 : b0 + w, :]
                )
                w *= 2
            nc.vector.tensor_mul(
                out=sb_o[b0 : b0 + 64, t, :],
                in0=sb_or[b0 : b0 + 64, t, :],
                in1=rbc[b0 : b0 + 64, :],
            )
            return
        rrow = wk.tile([1, S], bf16, tag="rrow", name=f"rrow{h}", bufs=2)
        nc.sync.dma_start(out=rrow[:, :], in_=rcp_sh[:, h, :])
        pbc = ppo.tile([64, S], f32, tag="ov", name=f"pbc{h}")
        for nn in range(2):
            nc.tensor.matmul(
                pbc[:, nn * 512 : (nn + 1) * 512],
                lhsT=ones_col[0:1, :],
                rhs=rrow[0:1, nn * 512 : (nn + 1) * 512],
                start=True,
                stop=True,
            )
        nc.vector.tensor_mul(
            out=sb_o[b0 : b0 + 64, t, :],
            in0=sb_or[b0 : b0 + 64, t, :],
            in1=pbc[:, :],
        )

    def proj_kk(psf, ob, kk):
        for nn in range(2):
            nc.tensor.matmul(
                psf[:, nn * 512 : (nn + 1) * 512],
                lhsT=(sb_wo[:, kk, ob * 128 : (ob + 1) * 128]),
                rhs=(sb_o[:, kk, nn * 512 : (nn + 1) * 512]),
                start=(kk == 0),
                stop=(kk == 3),
            )

    def proj_out(psf, ob):
        # quarter-granularity bias+store so the output DMAs overlap the
        # remaining ACT work in the epilogue
        for nn in range(2):
            ot = wk.tile([128, 512], f32, tag="ot", name=f"ot{ob}_{nn}", bufs=2)
            nc.scalar.activation(
                out=ot[:, :], in_=psf[:, nn * 512 : (nn + 1) * 512],
                func=AF.Identity, bias=sb_bo[:, ob : ob + 1],
            )
            nc.sync.dma_start(
                out=out_d[ob * 128 : (ob + 1) * 128, nn * 512 : (nn + 1) * 512],
                in_=ot[:, :],
            )

    def proj_t(t):
        """Output-projection partial for attention tile t, accumulated into
        SBUF f32 so the matmuls overlap attention instead of serializing at
        the tail.  t=0 evacuates via ACT with the bias fused; later t's add
        the PSUM partial on DVE; t=3 streams the result out."""
        for ob in range(2):
            psf = ppm.tile([128, S], f32, tag="mm", name=f"psf{ob}_{t}")
            for nn in range(2):
                nc.tensor.matmul(
                    psf[:, nn * 512 : (nn + 1) * 512],
                    lhsT=(sb_wo[:, t, ob * 128 : (ob + 1) * 128]),
                    rhs=(sb_o[:, t, nn * 512 : (nn + 1) * 512]),
                    start=True,
                    stop=True,
                )
            if t == 0:
                nc.scalar.activation(
                    out=acc[:, ob, :], in_=psf[:, :], func=AF.Identity,
                    bias=sb_bo[:, ob : ob + 1],
                )
            else:
                nc.vector.tensor_add(
                    out=acc[:, ob, :], in0=psf[:, :], in1=acc[:, ob, :]
                )
                if t == 3:
                    nc.sync.dma_start(
                        out=out_d[ob * 128 : (ob + 1) * 128, :], in_=acc[:, ob, :]
                    )

    # ---- emission order == scheduling priority ----
    qk_block(0)
    qk_block(4)
    rope_norm(0)
    for jb in range(8):
        v_block(jb)
    qk_block(1)
    qk_block(5)
    rope_norm(1)
    qk_block(2)
    qk_block(6)
    attn_head(0)
    rope_norm(2)
    attn_head(1)
    attn_norm(0)
    qk_block(3)
    qk_block(7)
    attn_head(2)
    attn_norm(1)
    rope_norm(3)
    attn_head(3)
    attn_norm(2)
    if TAILPROJ:
        attn_head(4)
        attn_norm(3)
        attn_head(5)
        attn_norm(4)
        attn_head(6)
        attn_norm(5)
        attn_head(7)
        # projection partials for the ready tiles run BEFORE the last two
        # normalize chains in the in-order PE queue; only kk=3 remains gated
        psf0 = ppm.tile([128, S], f32, tag="mm", name="psf0")
        psf1 = ppm.tile([128, S], f32, tag="mm", name="psf1")
        for kk in range(3):
            proj_kk(psf0, 0, kk)
            proj_kk(psf1, 1, kk)
        attn_norm(6)
        attn_norm(7)
        proj_kk(psf0, 0, 3)
        proj_kk(psf1, 1, 3)
        proj_out(psf0, 0)
        proj_out(psf1, 1)
    else:
        attn_head(4)
        attn_norm(3)
        attn_head(5)
        attn_norm(4)
        proj_t(0)
        attn_head(6)
        attn_norm(5)
        proj_t(1)
        attn_head(7)
        attn_norm(6)
        proj_t(2)
        attn_norm(7)
        proj_t(3)


def _patch_act_tables():
    """Steer the act-table-load pass to one set covering Exp+Ln+Copy.

    The default pass picks the first table set containing each activation
    function, which ping-pongs between exp_and_others and natural_log
    (~2.7us per reload).  Emptying every other set forces all activations
    onto natural_log_exp_and_others -> exactly one load.
    """
    import concourse.bacc as bacc

    if getattr(bacc, "_act_tables_patched", False):
        return
    import concourse.hw_specs as hw_specs

    orig = hw_specs.get_activation_tables

    def patched(arch):
        tables = orig(arch)
        keep = "natural_log_exp_and_others"
        assert keep in tables
        return {
            name: (fns if name == keep else set()) for name, fns in tables.items()
        }

    bacc.get_activation_tables = patched
    bacc._act_tables_patched = True


def _build():
    from contextlib import ExitStack

    import concourse.bacc as bacc
    import concourse.tile as tile
    from concourse import mybir

    _patch_act_tables()

    f32 = mybir.dt.float32
    bf16 = mybir.dt.bfloat16
    f8 = mybir.dt.float8e4
    nc = bacc.Bacc("TRN2", target_bir_lowering=False, debug=False, num_devices=N_CORES)
    aps = (
        nc.dram_tensor("x", [C, S], bf16, kind="ExternalInput").ap(),
        nc.dram_tensor("wqkT", [C, 2 * HID], bf16, kind="ExternalInput").ap(),
        nc.dram_tensor("x8", [128, 2, S], f8, kind="ExternalInput").ap(),
        nc.dram_tensor("wqk8", [128, 2, 2 * HID], f8, kind="ExternalInput").ap(),
        nc.dram_tensor("wvT", [C, HID], bf16, kind="ExternalInput").ap(),
        nc.dram_tensor("woT", [HID, C], bf16, kind="ExternalInput").ap(),
        nc.dram_tensor("bout", [128, 2], f32, kind="ExternalInput").ap(),
        nc.dram_tensor("cosT", [128, S], bf16, kind="ExternalInput").ap(),
        nc.dram_tensor("sinT", [128, S], bf16, kind="ExternalInput").ap(),
        nc.dram_tensor("out", [C, S], f32, kind="ExternalOutput").ap(),
    )
    with tile.TileContext(nc) as tc:
        with ExitStack() as ctx:
            _emit(ctx, tc, aps)
    nc.compile()
    return nc


def _get_nc():
    if "nc" not in _CACHE:
        _CACHE["nc"] = _build()
    return _CACHE["nc"]


def _make_in_maps(x, w_qkv, w_out, b_out):
    import ml_dtypes

    bf = ml_dtypes.bfloat16
    f8 = ml_dtypes.float8_e4m3
    xf32 = np.asarray(x, np.float32).reshape(N_CORES, C, S)
    xf = np.ascontiguousarray(xf32).astype(bf)
    wq = np.asarray(w_qkv, np.float32)
    wqkT = np.ascontiguousarray(wq[0 : 2 * HID, :].T).astype(bf)
    wvT = np.ascontiguousarray(wq[2 * HID : 3 * HID, :].T).astype(bf)
    x8 = np.ascontiguousarray(
        np.stack([xf32[:, 0:128, :], xf32[:, 128:256, :]], axis=2)
    ).astype(f8)
    wqkT256 = wq[0 : 2 * HID, :].T * 256.0
    wqk8 = np.ascontiguousarray(
        np.stack([wqkT256[0:128, :], wqkT256[128:256, :]], axis=1)
    ).astype(f8)
    woT = np.ascontiguousarray(np.asarray(w_out, np.float32).T).astype(bf)
    bo = np.ascontiguousarray(np.asarray(b_out, np.float32).reshape(2, 128).T)
    cosT, sinT = _rope_tables()
    shared = {
        "wqkT": wqkT,
        "wqk8": wqk8,
        "wvT": wvT,
        "woT": woT,
        "bout": bo,
        "cosT": cosT.astype(bf),
        "sinT": sinT.astype(bf),
    }
    return [
        dict(shared, x=np.ascontiguousarray(xf[i]), x8=np.ascontiguousarray(x8[i]))
        for i in range(N_CORES)
    ]


def _postprocess(res):
    out = np.stack([r["out"] for r in res.results], axis=0)
    return out.reshape(N_CORES, C, 32, 32).astype(np.float32)


def _run(x, w_qkv, w_out, b_out, trace=False):
    from concourse.bass_utils import run_bass_kernel_spmd

    nc = _get_nc()
    in_maps = _make_in_maps(x, w_qkv, w_out, b_out)
    res = run_bass_kernel_spmd(nc, in_maps, core_ids=list(range(N_CORES)), trace=trace)
    return _postprocess(res), res


def kernel(x, w_qkv, w_out, b_out):
    return _run(x, w_qkv, w_out, b_out, trace=False)[0]


# revision 69
# speedup vs baseline: 1.6645x; 1.0834x over previous
"""Trainium2 Bass kernel for AttentionWithRotaryPosEmb (8 cores, data-parallel).

Strategy
--------
Data-parallel over batch: each of the 8 NeuronCores computes one batch element
end-to-end. No collectives needed.

Per-core pipeline (batch x_b is [C=256, S=1024]):
  1. QKV projection: q,k as [o, s] (o = 64h+d), vT as [s, hid] straight out of
     the matmuls -- no transposes needed on-chip. PSUM evacuated by ACT copy
     (f32 -> bf16) so DVE stays free and later elementwise runs in bf16 DVE
     perf modes.
  2. RoPE on q,k in [d, s] layout: rotate-half is a single DVE stream_shuffle
     (32-partition-group permutation), then sin/cos multiplies + add against
     host-precomputed row tables (sign of sin encodes rotate-half; rows with
     d >= 32 have sin=0, cos=1).
  3. L2 norm over the sequence axis: square+reduce (ACT for tile 0 on the
     attention warm-up path, DVE after), rsqrt via exp(-0.5*ln(x)) on ACT --
     the activation-table pass is patched so Exp/Ln/Copy share one table set
     with the softmax exp (no table reloads). Both q,k scales fold into q.
  4. Attention with transposed softmax: simT[j, i] via matmul(lhsT=k_h,
     rhs=q_h); exp on ACT with scale=10 fused; softmax denominators come free
     from a ones-column appended to vT (65th lhsT column); outT[d, i]
     accumulates over j-blocks in PSUM.
  5. Softmax normalization: the unnormalized output parks in SBUF immediately
     (frees the PSUM ring); the denominator row is DMA-reshaped to [128, 8]
     so the DVE reciprocal runs multi-partition (0.3us instead of 6.5us
     single-lane), DMA'd back to a row, broadcast across 64 partitions by a
     K=1 PE matmul against a ones column (into a PSUM tile in the ppo pool --
     keeping it out of the ppm ring is critical, in-order PE queues otherwise
     stall the sim pipeline on the reciprocal roundtrip), then one DVE mul.
  6. Output projection accumulated per attention tile in PSUM: the kk=0..2
     partials run while the last two heads' normalize chains drain, only kk=3
     trails them; bias via ACT (per-partition bias AP) and quarter-granular
     stores overlap the epilogue.

  Notes: fp8/DoubleRow paths (K_FP8=1) are implemented but measurably SLOWER
  on this part -- the PE power governor throttles DoubleRow harder than the
  cycle savings.  tensor_tensor_reduce crashes at runtime on this hw/runtime
  combo; GPSIMD extended ops (partition_broadcast, swdge DMA) are unavailable
  (bedrock image, no HIPI ucode).  Exec time is throttle-lottery dependent:
  ~143-190us (median ambient ~150-170us) vs the 238us session baseline.
"""

import os
import sys

import numpy as np

if "/opt/trn_rl_repo" not in sys.path:
    sys.path.insert(0, "/opt/trn_rl_repo")

# bisect fallbacks (baseline-proven implementations).  tensor_tensor_reduce
# crashes at runtime on this hw/runtime combo -> always use mul+reduce_sum.
NO_SHUFFLE = os.environ.get("K_NO_SHUFFLE") == "1"
NO_TTR = True
NO_IDBIAS = os.environ.get("K_NO_IDBIAS") == "1"
NO_BCASTMM = os.environ.get("K_NO_BCASTMM") == "1"
TAILPROJ = os.environ.get("K_TAILPROJ", "1") == "1"
FP8 = os.environ.get("K_FP8") == "1"

HEADS = 8
DH = 64
S = 1024
C = 256
HID = 512
ROT = 32
HALF = 16
SCALE = 10.0
N_CORES = 8

# rotate-half as a 32-partition-group shuffle: swap the 16-row halves
SHUF_MASK = list(range(16, 32)) + list(range(16))

_CACHE = {}


def _rope_tables():
    """Row-patterned cos/sin tables [128, S] matching the q/k SBUF layout.

    Partition row r holds o-row (128t + r) of q/k tile t, i.e. head 2t + (r>=64)
    and d = r % 64.  Rows d in [0,16) get cos(i*invf[d]) / -sin(i*invf[d]);
    rows d in [16,32) get cos(i*invf[d-16]) / +sin(i*invf[d-16]); rows d >= 32
    get cos=1, sin=0 (identity).  The sign of sin encodes rotate_half.
    """
    inv = (
        1.0 / (np.float32(10000.0) ** (np.arange(0, ROT, 2, dtype=np.float32) / np.float32(ROT)))
    ).astype(np.float32)
    ang = (np.arange(S, dtype=np.float32)[None, :] * inv[:, None]).astype(np.float32)
    cos16 = np.cos(ang).astype(np.float32)  # [16, S]
    sin16 = np.sin(ang).astype(np.float32)
    cosT = np.ones((128, S), np.float32)
    sinT = np.zeros((128, S), np.float32)
    for r in range(128):
        d = r % 64
        if d < HALF:
            cosT[r] = cos16[d]
            sinT[r] = -sin16[d]
        elif d < ROT:
            cosT[r] = cos16[d - HALF]
            sinT[r] = sin16[d - HALF]
    return cosT, sinT


def _emit(ctx, tc, aps):
    import concourse.bass as bass  # noqa: F401
    from concourse import mybir

    f32 = mybir.dt.float32
    bf16 = mybir.dt.bfloat16
    f8 = mybir.dt.float8e4
    DR = mybir.MatmulPerfMode.DoubleRow
    AF = mybir.ActivationFunctionType
    ALU = mybir.AluOpType
    nc = tc.nc
    x_d, wqk_d, x8_d, wqk8_d, wv_d, wo_d, bo_d, cos_d, sin_d, out_d = aps

    singles = ctx.enter_context(tc.tile_pool(name="singles", bufs=1))
    wk = ctx.enter_context(tc.tile_pool(name="wk", bufs=3))
    ppm = ctx.enter_context(tc.tile_pool(name="ppm", bufs=2, space="PSUM"))
    ppo = ctx.enter_context(tc.tile_pool(name="ppo", bufs=2, space="PSUM"))

    # ---- persistent SBUF ----
    sb_x = singles.tile([128, 2, S], bf16)
    sb_wv = singles.tile([128, 2, HID], bf16)
    if FP8:
        sb_x8 = singles.tile([128, 2, S], f8)
        sb_wqk8 = singles.tile([128, 2, 2 * HID], f8)
        q8a = singles.tile([128, 4, S], f8)
        k8a = singles.tile([128, 4, S], f8)
        q8 = singles.tile([64, 4, 2, S], f8)
        k8 = singles.tile([64, 4, 2, S], f8)
        ln256 = singles.tile([128, 1], f32)
    else:
        sb_wqk = singles.tile([128, 2, 2 * HID], bf16)
    sb_wo = singles.tile([128, 4, C], bf16)
    sb_bo = singles.tile([128, 2], f32)
    sb_cos = singles.tile([128, S], bf16)
    sb_sin = singles.tile([128, S], bf16)
    sb_q = singles.tile([128, 4, S], bf16)   # raw q -> roped q (pre-scale)
    sb_k = singles.tile([128, 4, S], bf16)   # raw k (k*cos staging in-place)
    sb_qb = singles.tile([128, 4, S], bf16)  # normalized q, sim operand
    sb_kb = singles.tile([128, 4, S], bf16)  # roped k, sim operand
    sb_vx = singles.tile([128, 8, HEADS * (DH + 1)], bf16)
    sb_or = singles.tile([128, 4, S], bf16)  # attention out, unnormalized
    sb_o = singles.tile([128, 4, S], bf16)   # attention out, proj operand
    sb_stat = singles.tile([128, 16], f32)
    den_sh = singles.tile([128, 8, 8], f32)     # partition-spread denominators
    rcp_sh = singles.tile([128, 8, 8], bf16)    # reciprocals (bf16: bcast operand)
    ones_col = singles.tile([1, 64], bf16)      # lhsT for the rcp row broadcast
    acc = singles.tile([128, 2, S], f32)        # output-projection accumulator

    # ---- input loads (c-block granularity so QKV matmuls can start early) ----
    if FP8:
        nc.sync.dma_start(out=sb_wqk8[:, :, :], in_=wqk8_d[:, :])
        nc.sync.dma_start(out=sb_x8[:, :, :], in_=x8_d[:, :])
    else:
        nc.sync.dma_start(out=sb_wqk[:, 0, :], in_=wqk_d[0:128, :])
        nc.sync.dma_start(out=sb_wqk[:, 1, :], in_=wqk_d[128:256, :])
    nc.sync.dma_start(out=sb_x[:, 0, :], in_=x_d[0:128, :])
    nc.sync.dma_start(out=sb_x[:, 1, :], in_=x_d[128:256, :])
    nc.sync.dma_start(out=sb_wv[:, 0, :], in_=wv_d[0:128, :])
    nc.sync.dma_start(out=sb_wv[:, 1, :], in_=wv_d[128:256, :])
    nc.sync.dma_start(out=sb_cos[:, :], in_=cos_d[:, :])
    nc.sync.dma_start(out=sb_sin[:, :], in_=sin_d[:, :])
    for kk in range(4):
        nc.sync.dma_start(out=sb_wo[:, kk, :], in_=wo_d[kk * 128 : (kk + 1) * 128, :])
    nc.sync.dma_start(out=sb_bo[:, :], in_=bo_d[:, :])

    # ones column per head in vT_ext -> free softmax denominators
    vx4 = sb_vx.rearrange("p j (h e) -> p j h e", e=DH + 1)
    nc.vector.memset(vx4[:, :, :, DH : DH + 1], 1.0)
    nc.vector.memset(ones_col[:, :], 1.0)
    if FP8:
        nc.vector.memset(ln256[:, :], float(np.log(256.0)))

    def qk_block(ob):
        """q (ob<4) / k (ob>=4) projection o-block -> SBUF bf16 via ACT."""
        ps = ppm.tile([128, S], f32, tag="mm", name=f"ps_qk{ob}")
        for nn in range(2):
            if FP8:
                nc.tensor.matmul(
                    ps[:, nn * 512 : (nn + 1) * 512],
                    lhsT=(sb_wqk8[:, :, ob * 128 : (ob + 1) * 128]),
                    rhs=(sb_x8[:, :, nn * 512 : (nn + 1) * 512]),
                    start=True,
                    stop=True,
                    perf_mode=DR,
                )
                continue
            for kk in range(2):
                nc.tensor.matmul(
                    ps[:, nn * 512 : (nn + 1) * 512],
                    lhsT=(sb_wqk[:, kk, ob * 128 : (ob + 1) * 128]),
                    rhs=(sb_x[:, kk, nn * 512 : (nn + 1) * 512]),
                    start=(kk == 0),
                    stop=(kk == 1),
                )
        dst3 = sb_q if ob < 4 else sb_k
        nc.scalar.copy(out=dst3[:, ob % 4, :], in_=ps[:, :])

    def rope_norm(t):
        """RoPE + l2-norm stats for q/k tile t; fold both scales into q."""
        for src3, col in ((sb_q, 0), (sb_k, 4)):
            cur = src3[:, t, :]
            rot = wk.tile([128, S], bf16, tag="rot", name=f"rot{t}_{col}", bufs=2)
            # rotate-half: swap 16-row halves of each 32-partition group
            if NO_SHUFFLE:
                for base in range(0, 128, 32):
                    nc.sync.dma_start(
                        out=rot[base : base + 16, :], in_=src3[base + 16 : base + 32, t, :]
                    )
                    nc.sync.dma_start(
                        out=rot[base + 16 : base + 32, :], in_=src3[base : base + 16, t, :]
                    )
            else:
                nc.vector.stream_shuffle(out=rot[:, :], in_=cur, mask=SHUF_MASK)
            nc.vector.tensor_mul(out=rot[:, :], in0=rot[:, :], in1=sb_sin[:, :])
            nc.vector.tensor_mul(out=cur, in0=cur, in1=sb_cos[:, :])
            adddst = cur if col == 0 else sb_kb[:, t, :]
            nc.vector.tensor_add(out=adddst, in0=cur, in1=rot[:, :])
            # fused square + sum over s -> stat col.  Tile 0 is the attention
            # warm-up critical path: run it on ACT (idle then) to shorten the
            # DVE chain; ACT is exp-saturated for later tiles.
            sq = wk.tile([128, S], bf16, tag="sq", name=f"sq{t}_{col}", bufs=2)
            if t == 0:
                nc.scalar.activation(
                    out=sq[:, :], in_=adddst, func=AF.Square,
                    accum_out=sb_stat[:, col + t : col + t + 1],
                )
            elif NO_TTR:
                nc.vector.tensor_mul(out=sq[:, :], in0=adddst, in1=adddst)
                nc.vector.reduce_sum(
                    out=sb_stat[:, col + t : col + t + 1], in_=sq[:, :],
                    axis=mybir.AxisListType.X,
                )
            else:
                nc.vector.tensor_tensor_reduce(
                    out=sq[:, :], in0=adddst, in1=adddst, scale=1.0, scalar=0.0,
                    op0=ALU.mult, op1=ALU.add,
                    accum_out=sb_stat[:, col + t : col + t + 1],
                )
        # rs_comb = (ssq_q * ssq_k)^-1/2 = exp(-0.5*(ln q + ln k)); the x256
        # from the fp8 qk weights cancels (sim is scale-invariant through it)
        nc.scalar.activation(
            out=sb_stat[:, 8 + t : 9 + t], in_=sb_stat[:, t : t + 1],
            func=AF.Ln, bias=0.0,
        )
        nc.scalar.activation(
            out=sb_stat[:, 12 + t : 13 + t], in_=sb_stat[:, 4 + t : 5 + t],
            func=AF.Ln, bias=0.0,
        )
        nc.vector.tensor_add(
            out=sb_stat[:, 8 + t : 9 + t],
            in0=sb_stat[:, 8 + t : 9 + t],
            in1=sb_stat[:, 12 + t : 13 + t],
        )
        if FP8:
            nc.scalar.activation(
                out=sb_stat[:, 12 + t : 13 + t], in_=sb_stat[:, 8 + t : 9 + t],
                func=AF.Exp, scale=-0.25, bias=ln256[:, 0:1],
            )
            sc_col = sb_stat[:, 12 + t : 13 + t]
            nc.vector.tensor_scalar_mul(out=q8a[:, t, :], in0=sb_q[:, t, :], scalar1=sc_col)
            nc.vector.tensor_scalar_mul(out=k8a[:, t, :], in0=sb_kb[:, t, :], scalar1=sc_col)
            for src8, dst8 in ((q8a, q8), (k8a, k8)):
                for hh in range(2):
                    for i in range(2):
                        nc.sync.dma_start(
                            out=dst8[32 * hh : 32 * hh + 32, t, i, :],
                            in_=src8[64 * hh + 32 * i : 64 * hh + 32 * i + 32, t, :],
                        )
        else:
            nc.scalar.activation(
                out=sb_stat[:, 12 + t : 13 + t], in_=sb_stat[:, 8 + t : 9 + t],
                func=AF.Exp, scale=-0.5,
            )
            nc.vector.tensor_scalar_mul(
                out=sb_qb[:, t, :], in0=sb_q[:, t, :],
                scalar1=sb_stat[:, 12 + t : 13 + t],
            )

    def v_block(jb):
        """vT s-block: [s_jb, hid] straight from matmul, strided into vx ext."""
        psv = ppo.tile([128, 512], f32, tag="ov", name=f"psv{jb}")
        for kk in range(2):
            nc.tensor.matmul(
                psv[:, :],
                lhsT=(sb_x[:, kk, jb * 128 : (jb + 1) * 128]),
                rhs=(sb_wv[:, kk, :]),
                start=(kk == 0),
                stop=(kk == 1),
            )
        dst = sb_vx[:, jb, :].rearrange("p (h e) -> p h e", e=DH + 1)[:, :, 0:DH]
        src = psv.rearrange("p (h d) -> p h d", d=DH)
        nc.vector.tensor_copy(out=dst, in_=src)  # f32 PSUM -> bf16 SBUF cast

    def attn_head(h):
        t, half = h // 2, h % 2
        b0 = 64 * half
        b1 = 32 * half
        pso = ppo.tile([65, S], f32, tag="ov", name=f"pso{h}")
        for jb in range(8):
            pss = ppm.tile([128, S], f32, tag="mm", name=f"pss{h}_{jb}")
            for nn in range(2):
                if FP8:
                    nc.tensor.matmul(
                        pss[:, nn * 512 : (nn + 1) * 512],
                        lhsT=(k8[b1 : b1 + 32, t, :, jb * 128 : (jb + 1) * 128]),
                        rhs=(q8[b1 : b1 + 32, t, :, nn * 512 : (nn + 1) * 512]),
                        start=True,
                        stop=True,
                        perf_mode=DR,
                    )
                else:
                    nc.tensor.matmul(
                        pss[:, nn * 512 : (nn + 1) * 512],
                        lhsT=(sb_kb[b0 : b0 + 64, t, jb * 128 : (jb + 1) * 128]),
                        rhs=(sb_qb[b0 : b0 + 64, t, nn * 512 : (nn + 1) * 512]),
                        start=True,
                        stop=True,
                    )
            et = wk.tile([128, S], bf16, tag="et", name=f"et{h}_{jb}", bufs=3)
            nc.scalar.activation(
                out=et[:, :], in_=pss[:, :], func=AF.Exp,
                scale=(SCALE / 65536.0 if FP8 else SCALE),
            )
            for nn in range(2):
                nc.tensor.matmul(
                    pso[0:65, nn * 512 : (nn + 1) * 512],
                    lhsT=(sb_vx[:, jb, h * (DH + 1) : (h + 1) * (DH + 1)]),
                    rhs=(et[:, nn * 512 : (nn + 1) * 512]),
                    start=(jb == 0),
                    stop=(jb == 7),
                )
        # park the unnormalized output + denominator row; frees the PSUM buf
        # without waiting for the reciprocal roundtrip.  For the last head the
        # output parks via ACT (idle once the final exp retires), halving the
        # tail's serial DVE chain.
        if h == HEADS - 1:
            nc.scalar.copy(out=sb_or[b0 : b0 + 64, t, :], in_=pso[0:64, :])
        else:
            nc.vector.tensor_copy(out=sb_or[b0 : b0 + 64, t, :], in_=pso[0:64, :])
        dstage = wk.tile([1, S], f32, tag="dstage", name=f"dstage{h}", bufs=2)
        nc.vector.tensor_copy(out=dstage[:, :], in_=pso[64:65, :])
        nc.sync.dma_start(out=den_sh[:, h, :], in_=dstage[:, :])
        with nc.allow_low_precision("softmax denominator reciprocal in bf16"):
            nc.vector.reciprocal(out=rcp_sh[:, h, :], in_=den_sh[:, h, :])

    def attn_norm(h):
        """Broadcast 1/den across partitions via a K=1 PE matmul, then scale.

        Emitted one head late so the PE queue never stalls on the reciprocal
        roundtrip; the broadcast tile borrows a ppm slot (PSUM is full during
        attention).
        """
        t, half = h // 2, h % 2
        b0 = 64 * half
        if NO_BCASTMM:
            # both-SBUF tensor_tensor needs equal base partitions: use a full
            # [128, S] tile and fill rows b0..b0+64 via the doubling chain
            rbc = wk.tile([128, S], bf16, tag="rbc", name=f"rbc{h}", bufs=2)
            nc.sync.dma_start(out=rbc[b0 : b0 + 1, :], in_=rcp_sh[:, h, :])
            w = 1
            while w < 64:
                nc.sync.dma_start(
                    out=rbc[b0 + w : b0 + 2 * w, :], in_=rbc[b0 : b0 + w, :]
                )
                w *= 2
            nc.vector.tensor_mul(
                out=sb_o[b0 : b0 + 64, t, :],
                in0=sb_or[b0 : b0 + 64, t, :],
                in1=rbc[b0 : b0 + 64, :],
            )
            return
        rrow = wk.tile([1, S], bf16, tag="rrow", name=f"rrow{h}", bufs=2)
        nc.sync.dma_start(out=rrow[:, :], in_=rcp_sh[:, h, :])
        pbc = ppo.tile([64, S], f32, tag="ov", name=f"pbc{h}")
        for nn in range(2):
            nc.tensor.matmul(
                pbc[:, nn * 512 : (nn + 1) * 512],
                lhsT=ones_col[0:1, :],
                rhs=rrow[0:1, nn * 512 : (nn + 1) * 512],
                start=True,
                stop=True,
            )
        nc.vector.tensor_mul(
            out=sb_o[b0 : b0 + 64, t, :],
            in0=sb_or[b0 : b0 + 64, t, :],
            in1=pbc[:, :],
        )

    def proj_kk(psf, ob, kk):
        for nn in range(2):
            nc.tensor.matmul(
                psf[:, nn * 512 : (nn + 1) * 512],
                lhsT=(sb_wo[:, kk, ob * 128 : (ob + 1) * 128]),
                rhs=(sb_o[:, kk, nn * 512 : (nn + 1) * 512]),
                start=(kk == 0),
                stop=(kk == 3),
            )

    def proj_out(psf, ob):
        # quarter-granularity bias+store so the output DMAs overlap the
        # remaining ACT work in the epilogue
        for nn in range(2):
            ot = wk.tile([128, 512], f32, tag="ot", name=f"ot{ob}_{nn}", bufs=2)
            nc.scalar.activation(
                out=ot[:, :], in_=psf[:, nn * 512 : (nn + 1) * 512],
                func=AF.Identity, bias=sb_bo[:, ob : ob + 1],
            )
            nc.sync.dma_start(
                out=out_d[ob * 128 : (ob + 1) * 128, nn * 512 : (nn + 1) * 512],
                in_=ot[:, :],
            )

    def proj_t(t):
        """Output-projection partial for attention tile t, accumulated into
        SBUF f32 so the matmuls overlap attention instead of serializing at
        the tail.  t=0 evacuates via ACT with the bias fused; later t's add
        the PSUM partial on DVE; t=3 streams the result out."""
        for ob in range(2):
            psf = ppm.tile([128, S], f32, tag="mm", name=f"psf{ob}_{t}")
            for nn in range(2):
                nc.tensor.matmul(
                    psf[:, nn * 512 : (nn + 1) * 512],
                    lhsT=(sb_wo[:, t, ob * 128 : (ob + 1) * 128]),
                    rhs=(sb_o[:, t, nn * 512 : (nn + 1) * 512]),
                    start=True,
                    stop=True,
                )
            if t == 0:
                nc.scalar.activation(
                    out=acc[:, ob, :], in_=psf[:, :], func=AF.Identity,
                    bias=sb_bo[:, ob : ob + 1],
                )
            else:
                nc.vector.tensor_add(
                    out=acc[:, ob, :], in0=psf[:, :], in1=acc[:, ob, :]
                )
                if t == 3:
                    nc.sync.dma_start(
                        out=out_d[ob * 128 : (ob + 1) * 128, :], in_=acc[:, ob, :]
                    )

    # ---- emission order == scheduling priority ----
    qk_block(0)
    qk_block(4)
    rope_norm(0)
    for jb in range(8):
        v_block(jb)
    qk_block(1)
    qk_block(5)
    rope_norm(1)
    qk_block(2)
    qk_block(6)
    attn_head(0)
    rope_norm(2)
    attn_head(1)
    attn_norm(0)
    qk_block(3)
    qk_block(7)
    attn_head(2)
    attn_norm(1)
    rope_norm(3)
    attn_head(3)
    attn_norm(2)
    if TAILPROJ:
        attn_head(4)
        attn_norm(3)
        attn_head(5)
        attn_norm(4)
        attn_head(6)
        attn_norm(5)
        attn_head(7)
        # projection partials for the ready tiles run BEFORE the last two
        # normalize chains in the in-order PE queue; only kk=3 remains gated
        psf0 = ppm.tile([128, S], f32, tag="mm", name="psf0")
        psf1 = ppm.tile([128, S], f32, tag="mm", name="psf1")
        for kk in range(3):
            proj_kk(psf0, 0, kk)
            proj_kk(psf1, 1, kk)
        attn_norm(6)
        attn_norm(7)
        proj_kk(psf0, 0, 3)
        proj_kk(psf1, 1, 3)
        proj_out(psf0, 0)
        proj_out(psf1, 1)
    else:
        attn_head(4)
        attn_norm(3)
        attn_head(5)
        attn_norm(4)
        proj_t(0)
        attn_head(6)
        attn_norm(5)
        proj_t(1)
        attn_head(7)
        attn_norm(6)
        proj_t(2)
        attn_norm(7)
        proj_t(3)


def _patch_act_tables():
    """Steer the act-table-load pass to one set covering Exp+Ln+Copy.

    The default pass picks the first table set containing each activation
    function, which ping-pongs between exp_and_others and natural_log
    (~2.7us per reload).  Emptying every other set forces all activations
    onto natural_log_exp_and_others -> exactly one load.
    """
    import concourse.bacc as bacc

    if getattr(bacc, "_act_tables_patched", False):
        return
    import concourse.hw_specs as hw_specs

    orig = hw_specs.get_activation_tables

    def patched(arch):
        tables = orig(arch)
        keep = "natural_log_exp_and_others"
        assert keep in tables
        return {
            name: (fns if name == keep else set()) for name, fns in tables.items()
        }

    bacc.get_activation_tables = patched
    bacc._act_tables_patched = True


def _build():
    from contextlib import ExitStack

    import concourse.bacc as bacc
    import concourse.tile as tile
    from concourse import mybir

    _patch_act_tables()

    f32 = mybir.dt.float32
    bf16 = mybir.dt.bfloat16
    f8 = mybir.dt.float8e4
    nc = bacc.Bacc("TRN2", target_bir_lowering=False, debug=False, num_devices=N_CORES)
    aps = (
        nc.dram_tensor("x", [C, S], bf16, kind="ExternalInput").ap(),
        nc.dram_tensor("wqkT", [C, 2 * HID], bf16, kind="ExternalInput").ap(),
        nc.dram_tensor("x8", [128, 2, S], f8, kind="ExternalInput").ap(),
        nc.dram_tensor("wqk8", [128, 2, 2 * HID], f8, kind="ExternalInput").ap(),
        nc.dram_tensor("wvT", [C, HID], bf16, kind="ExternalInput").ap(),
        nc.dram_tensor("woT", [HID, C], bf16, kind="ExternalInput").ap(),
        nc.dram_tensor("bout", [128, 2], f32, kind="ExternalInput").ap(),
        nc.dram_tensor("cosT", [128, S], bf16, kind="ExternalInput").ap(),
        nc.dram_tensor("sinT", [128, S], bf16, kind="ExternalInput").ap(),
        nc.dram_tensor("out", [C, S], f32, kind="ExternalOutput").ap(),
    )
    with tile.TileContext(nc) as tc:
        with ExitStack() as ctx:
            _emit(ctx, tc, aps)
    nc.compile()
    return nc


def _get_nc():
    if "nc" not in _CACHE:
        _CACHE["nc"] = _build()
    return _CACHE["nc"]


def _make_in_maps(x, w_qkv, w_out, b_out):
    import ml_dtypes

    bf = ml_dtypes.bfloat16
    f8 = ml_dtypes.float8_e4m3
    xf32 = np.asarray(x, np.float32).reshape(N_CORES, C, S)
    xf = np.ascontiguousarray(xf32).astype(bf)
    wq = np.asarray(w_qkv, np.float32)
    wqkT = np.ascontiguousarray(wq[0 : 2 * HID, :].T).astype(bf)
    wvT = np.ascontiguousarray(wq[2 * HID : 3 * HID, :].T).astype(bf)
    x8 = np.ascontiguousarray(
        np.stack([xf32[:, 0:128, :], xf32[:, 128:256, :]], axis=2)
    ).astype(f8)
    wqkT256 = wq[0 : 2 * HID, :].T * 256.0
    wqk8 = np.ascontiguousarray(
        np.stack([wqkT256[0:128, :], wqkT256[128:256, :]], axis=1)
    ).astype(f8)
    woT = np.ascontiguousarray(np.asarray(w_out, np.float32).T).astype(bf)
    bo = np.ascontiguousarray(np.asarray(b_out, np.float32).reshape(2, 128).T)
    cosT, sinT = _rope_tables()
    shared = {
        "wqkT": wqkT,
        "wqk8": wqk8,
        "wvT": wvT,
        "woT": woT,
        "bout": bo,
        "cosT": cosT.astype(bf),
        "sinT": sinT.astype(bf),
    }
    return [
        dict(shared, x=np.ascontiguousarray(xf[i]), x8=np.ascontiguousarray(x8[i]))
        for i in range(N_CORES)
    ]


def _postprocess(res):
    out = np.stack([r["out"] for r in res.results], axis=0)
    return out.reshape(N_CORES, C, 32, 32).astype(np.float32)


def _run(x, w_qkv, w_out, b_out, trace=False):
    from concourse.bass_utils import run_bass_kernel_spmd

    nc = _get_nc()
    in_maps = _make_in_maps(x, w_qkv, w_out, b_out)
    res = run_bass_kernel_spmd(nc, in_maps, core_ids=list(range(N_CORES)), trace=trace)
    return _postprocess(res), res


def kernel(x, w_qkv, w_out, b_out):
    return _run(x, w_qkv, w_out, b_out, trace=False)[0]


# revision 74
# speedup vs baseline: 1.6680x; 1.0021x over previous
"""Trainium2 Bass kernel for AttentionWithRotaryPosEmb (8 cores, data-parallel).

Strategy
--------
Data-parallel over batch: each of the 8 NeuronCores computes one batch element
end-to-end. No collectives needed.

Per-core pipeline (batch x_b is [C=256, S=1024]):
  1. QKV projection: q,k as [o, s] (o = 64h+d), vT as [s, hid] straight out of
     the matmuls -- no transposes needed on-chip. PSUM evacuated by ACT copy
     (f32 -> bf16) so DVE stays free and later elementwise runs in bf16 DVE
     perf modes.
  2. RoPE on q,k in [d, s] layout: rotate-half is a single DVE stream_shuffle
     (32-partition-group permutation), then sin/cos multiplies + add against
     host-precomputed row tables (sign of sin encodes rotate-half; rows with
     d >= 32 have sin=0, cos=1).
  3. L2 norm over the sequence axis: square+reduce (ACT for tile 0 on the
     attention warm-up path, DVE after), rsqrt via exp(-0.5*ln(x)) on ACT --
     the activation-table pass is patched so Exp/Ln/Copy share one table set
     with the softmax exp (no table reloads). Both q,k scales fold into q.
  4. Attention with transposed softmax: simT[j, i] via matmul(lhsT=k_h,
     rhs=q_h); exp on ACT with scale=10 fused; softmax denominators come free
     from a ones-column appended to vT (65th lhsT column); outT[d, i]
     accumulates over j-blocks in PSUM.
  5. Softmax normalization: the unnormalized output parks in SBUF immediately
     (frees the PSUM ring); the denominator row is DMA-reshaped to [128, 8]
     so the DVE reciprocal runs multi-partition (0.3us instead of 6.5us
     single-lane), DMA'd back to a row, broadcast across 64 partitions by a
     K=1 PE matmul against a ones column (into a PSUM tile in the ppo pool --
     keeping it out of the ppm ring is critical, in-order PE queues otherwise
     stall the sim pipeline on the reciprocal roundtrip), then one DVE mul.
  6. Output projection accumulated per attention tile in PSUM: the kk=0..2
     partials run while the last two heads' normalize chains drain, only kk=3
     trails them; bias via ACT (per-partition bias AP) and quarter-granular
     stores overlap the epilogue.

  Notes: fp8/DoubleRow paths (K_FP8=1) are implemented but measurably SLOWER
  on this part -- the PE power governor throttles DoubleRow harder than the
  cycle savings.  tensor_tensor_reduce crashes at runtime on this hw/runtime
  combo; GPSIMD extended ops (partition_broadcast, swdge DMA) are unavailable
  (bedrock image, no HIPI ucode).  Exec time is throttle-lottery dependent:
  ~143-190us (median ambient ~150-170us) vs the 238us session baseline.
"""

import os
import sys

import numpy as np

if "/opt/trn_rl_repo" not in sys.path:
    sys.path.insert(0, "/opt/trn_rl_repo")

# bisect fallbacks (baseline-proven implementations).  tensor_tensor_reduce
# crashes at runtime on this hw/runtime combo -> always use mul+reduce_sum.
NO_SHUFFLE = os.environ.get("K_NO_SHUFFLE") == "1"
NO_TTR = True
NO_IDBIAS = os.environ.get("K_NO_IDBIAS") == "1"
NO_BCASTMM = os.environ.get("K_NO_BCASTMM") == "1"
TAILPROJ = os.environ.get("K_TAILPROJ", "1") == "1"
FP8 = os.environ.get("K_FP8") == "1"

HEADS = 8
DH = 64
S = 1024
C = 256
HID = 512
ROT = 32
HALF = 16
SCALE = 10.0
N_CORES = 8

# rotate-half as a 32-partition-group shuffle: swap the 16-row halves
SHUF_MASK = list(range(16, 32)) + list(range(16))

_CACHE = {}


def _rope_tables():
    """Row-patterned cos/sin tables [128, S] matching the q/k SBUF layout.

    Partition row r holds o-row (128t + r) of q/k tile t, i.e. head 2t + (r>=64)
    and d = r % 64.  Rows d in [0,16) get cos(i*invf[d]) / -sin(i*invf[d]);
    rows d in [16,32) get cos(i*invf[d-16]) / +sin(i*invf[d-16]); rows d >= 32
    get cos=1, sin=0 (identity).  The sign of sin encodes rotate_half.
    """
    inv = (
        1.0 / (np.float32(10000.0) ** (np.arange(0, ROT, 2, dtype=np.float32) / np.float32(ROT)))
    ).astype(np.float32)
    ang = (np.arange(S, dtype=np.float32)[None, :] * inv[:, None]).astype(np.float32)
    cos16 = np.cos(ang).astype(np.float32)  # [16, S]
    sin16 = np.sin(ang).astype(np.float32)
    cosT = np.ones((128, S), np.float32)
    sinT = np.zeros((128, S), np.float32)
    for r in range(128):
        d = r % 64
        if d < HALF:
            cosT[r] = cos16[d]
            sinT[r] = -sin16[d]
        elif d < ROT:
            cosT[r] = cos16[d - HALF]
            sinT[r] = sin16[d - HALF]
    return cosT, sinT


def _emit(ctx, tc, aps):
    import concourse.bass as bass  # noqa: F401
    from concourse import mybir

    f32 = mybir.dt.float32
    bf16 = mybir.dt.bfloat16
    f8 = mybir.dt.float8e4
    DR = mybir.MatmulPerfMode.DoubleRow
    AF = mybir.ActivationFunctionType
    ALU = mybir.AluOpType
    nc = tc.nc
    x_d, wqk_d, x8_d, wqk8_d, wv_d, wo_d, bo_d, cos_d, sin_d, out_d = aps

    singles = ctx.enter_context(tc.tile_pool(name="singles", bufs=1))
    wk = ctx.enter_context(tc.tile_pool(name="wk", bufs=3))
    ppm = ctx.enter_context(tc.tile_pool(name="ppm", bufs=2, space="PSUM"))
    ppo = ctx.enter_context(tc.tile_pool(name="ppo", bufs=2, space="PSUM"))

    # ---- persistent SBUF ----
    sb_x = singles.tile([128, 2, S], bf16)
    sb_wv = singles.tile([128, 2, HID], bf16)
    if FP8:
        sb_x8 = singles.tile([128, 2, S], f8)
        sb_wqk8 = singles.tile([128, 2, 2 * HID], f8)
        q8a = singles.tile([128, 4, S], f8)
        k8a = singles.tile([128, 4, S], f8)
        q8 = singles.tile([64, 4, 2, S], f8)
        k8 = singles.tile([64, 4, 2, S], f8)
        ln256 = singles.tile([128, 1], f32)
    else:
        sb_wqk = singles.tile([128, 2, 2 * HID], bf16)
    sb_wo = singles.tile([128, 4, C], bf16)
    sb_bo = singles.tile([128, 2], f32)
    sb_cos = singles.tile([128, S], bf16)
    sb_sin = singles.tile([128, S], bf16)
    sb_q = singles.tile([128, 4, S], bf16)   # raw q -> roped q (pre-scale)
    sb_k = singles.tile([128, 4, S], bf16)   # raw k (k*cos staging in-place)
    sb_qb = singles.tile([128, 4, S], bf16)  # normalized q, sim operand
    sb_kb = singles.tile([128, 4, S], bf16)  # roped k, sim operand
    sb_vx = singles.tile([128, 8, HEADS * (DH + 1)], bf16)
    sb_or = singles.tile([128, 4, S], bf16)  # attention out, unnormalized
    sb_o = singles.tile([128, 4, S], bf16)   # attention out, proj operand
    sb_stat = singles.tile([128, 16], f32)
    den_sh = singles.tile([128, 8, 8], f32)     # partition-spread denominators
    rcp_sh = singles.tile([128, 8, 8], bf16)    # reciprocals (bf16: bcast operand)
    ones_col = singles.tile([1, 64], bf16)      # lhsT for the rcp row broadcast
    acc = singles.tile([128, 2, S], f32)        # output-projection accumulator

    # ---- input loads (c-block granularity so QKV matmuls can start early) ----
    if FP8:
        nc.sync.dma_start(out=sb_wqk8[:, :, :], in_=wqk8_d[:, :])
        nc.sync.dma_start(out=sb_x8[:, :, :], in_=x8_d[:, :])
    else:
        nc.sync.dma_start(out=sb_wqk[:, 0, :], in_=wqk_d[0:128, :])
        nc.sync.dma_start(out=sb_wqk[:, 1, :], in_=wqk_d[128:256, :])
    nc.sync.dma_start(out=sb_x[:, 0, :], in_=x_d[0:128, :])
    nc.sync.dma_start(out=sb_x[:, 1, :], in_=x_d[128:256, :])
    nc.sync.dma_start(out=sb_wv[:, 0, :], in_=wv_d[0:128, :])
    nc.sync.dma_start(out=sb_wv[:, 1, :], in_=wv_d[128:256, :])
    nc.sync.dma_start(out=sb_cos[:, :], in_=cos_d[:, :])
    nc.sync.dma_start(out=sb_sin[:, :], in_=sin_d[:, :])
    for kk in range(4):
        nc.sync.dma_start(out=sb_wo[:, kk, :], in_=wo_d[kk * 128 : (kk + 1) * 128, :])
    nc.sync.dma_start(out=sb_bo[:, :], in_=bo_d[:, :])

    # ones column per head in vT_ext -> free softmax denominators
    vx4 = sb_vx.rearrange("p j (h e) -> p j h e", e=DH + 1)
    nc.vector.memset(vx4[:, :, :, DH : DH + 1], 1.0)
    nc.vector.memset(ones_col[:, :], 1.0)
    if FP8:
        nc.vector.memset(ln256[:, :], float(np.log(256.0)))

    def qk_block(ob):
        """q (ob<4) / k (ob>=4) projection o-block -> SBUF bf16 via ACT."""
        ps = ppm.tile([128, S], f32, tag="mm", name=f"ps_qk{ob}")
        for nn in range(2):
            if FP8:
                nc.tensor.matmul(
                    ps[:, nn * 512 : (nn + 1) * 512],
                    lhsT=(sb_wqk8[:, :, ob * 128 : (ob + 1) * 128]),
                    rhs=(sb_x8[:, :, nn * 512 : (nn + 1) * 512]),
                    start=True,
                    stop=True,
                    perf_mode=DR,
                )
                continue
            for kk in range(2):
                nc.tensor.matmul(
                    ps[:, nn * 512 : (nn + 1) * 512],
                    lhsT=(sb_wqk[:, kk, ob * 128 : (ob + 1) * 128]),
                    rhs=(sb_x[:, kk, nn * 512 : (nn + 1) * 512]),
                    start=(kk == 0),
                    stop=(kk == 1),
                )
        dst3 = sb_q if ob < 4 else sb_k
        nc.scalar.copy(out=dst3[:, ob % 4, :], in_=ps[:, :])

    def rope_norm(t):
        """RoPE + l2-norm stats for q/k tile t; fold both scales into q."""
        for src3, col in ((sb_q, 0), (sb_k, 4)):
            cur = src3[:, t, :]
            rot = wk.tile([128, S], bf16, tag="rot", name=f"rot{t}_{col}", bufs=2)
            # rotate-half: swap 16-row halves of each 32-partition group
            if NO_SHUFFLE:
                for base in range(0, 128, 32):
                    nc.sync.dma_start(
                        out=rot[base : base + 16, :], in_=src3[base + 16 : base + 32, t, :]
                    )
                    nc.sync.dma_start(
                        out=rot[base + 16 : base + 32, :], in_=src3[base : base + 16, t, :]
                    )
            else:
                nc.vector.stream_shuffle(out=rot[:, :], in_=cur, mask=SHUF_MASK)
            nc.vector.tensor_mul(out=rot[:, :], in0=rot[:, :], in1=sb_sin[:, :])
            nc.vector.tensor_mul(out=cur, in0=cur, in1=sb_cos[:, :])
            adddst = cur if col == 0 else sb_kb[:, t, :]
            nc.vector.tensor_add(out=adddst, in0=cur, in1=rot[:, :])
            # fused square + sum over s -> stat col.  Tile 0 is the attention
            # warm-up critical path: run it on ACT (idle then) to shorten the
            # DVE chain; ACT is exp-saturated for later tiles.
            sq = wk.tile([128, S], bf16, tag="sq", name=f"sq{t}_{col}", bufs=2)
            if t == 0:
                nc.scalar.activation(
                    out=sq[:, :], in_=adddst, func=AF.Square,
                    accum_out=sb_stat[:, col + t : col + t + 1],
                )
            elif NO_TTR:
                nc.vector.tensor_mul(out=sq[:, :], in0=adddst, in1=adddst)
                nc.vector.reduce_sum(
                    out=sb_stat[:, col + t : col + t + 1], in_=sq[:, :],
                    axis=mybir.AxisListType.X,
                )
            else:
                nc.vector.tensor_tensor_reduce(
                    out=sq[:, :], in0=adddst, in1=adddst, scale=1.0, scalar=0.0,
                    op0=ALU.mult, op1=ALU.add,
                    accum_out=sb_stat[:, col + t : col + t + 1],
                )
        # rs_comb = (ssq_q * ssq_k)^-1/2 = exp(-0.5*(ln q + ln k)); the x256
        # from the fp8 qk weights cancels (sim is scale-invariant through it)
        nc.scalar.activation(
            out=sb_stat[:, 8 + t : 9 + t], in_=sb_stat[:, t : t + 1],
            func=AF.Ln, bias=0.0,
        )
        nc.scalar.activation(
            out=sb_stat[:, 12 + t : 13 + t], in_=sb_stat[:, 4 + t : 5 + t],
            func=AF.Ln, bias=0.0,
        )
        nc.vector.tensor_add(
            out=sb_stat[:, 8 + t : 9 + t],
            in0=sb_stat[:, 8 + t : 9 + t],
            in1=sb_stat[:, 12 + t : 13 + t],
        )
        if FP8:
            nc.scalar.activation(
                out=sb_stat[:, 12 + t : 13 + t], in_=sb_stat[:, 8 + t : 9 + t],
                func=AF.Exp, scale=-0.25, bias=ln256[:, 0:1],
            )
            sc_col = sb_stat[:, 12 + t : 13 + t]
            nc.vector.tensor_scalar_mul(out=q8a[:, t, :], in0=sb_q[:, t, :], scalar1=sc_col)
            nc.vector.tensor_scalar_mul(out=k8a[:, t, :], in0=sb_kb[:, t, :], scalar1=sc_col)
            for src8, dst8 in ((q8a, q8), (k8a, k8)):
                for hh in range(2):
                    for i in range(2):
                        nc.sync.dma_start(
                            out=dst8[32 * hh : 32 * hh + 32, t, i, :],
                            in_=src8[64 * hh + 32 * i : 64 * hh + 32 * i + 32, t, :],
                        )
        else:
            nc.scalar.activation(
                out=sb_stat[:, 12 + t : 13 + t], in_=sb_stat[:, 8 + t : 9 + t],
                func=AF.Exp, scale=-0.5,
            )
            nc.vector.tensor_scalar_mul(
                out=sb_qb[:, t, :], in0=sb_q[:, t, :],
                scalar1=sb_stat[:, 12 + t : 13 + t],
            )

    def v_block(jb):
        """vT s-block: [s_jb, hid] straight from matmul, strided into vx ext."""
        psv = ppo.tile([128, 512], f32, tag="ov", name=f"psv{jb}")
        for kk in range(2):
            nc.tensor.matmul(
                psv[:, :],
                lhsT=(sb_x[:, kk, jb * 128 : (jb + 1) * 128]),
                rhs=(sb_wv[:, kk, :]),
                start=(kk == 0),
                stop=(kk == 1),
            )
        dst = sb_vx[:, jb, :].rearrange("p (h e) -> p h e", e=DH + 1)[:, :, 0:DH]
        src = psv.rearrange("p (h d) -> p h d", d=DH)
        nc.vector.tensor_copy(out=dst, in_=src)  # f32 PSUM -> bf16 SBUF cast

    def attn_head(h):
        t, half = h // 2, h % 2
        b0 = 64 * half
        b1 = 32 * half
        pso = ppo.tile([65, S], f32, tag="ov", name=f"pso{h}")

        def av(jb, et):
            for nn in range(2):
                nc.tensor.matmul(
                    pso[0:65, nn * 512 : (nn + 1) * 512],
                    lhsT=(sb_vx[:, jb, h * (DH + 1) : (h + 1) * (DH + 1)]),
                    rhs=(et[:, nn * 512 : (nn + 1) * 512]),
                    start=(jb == 0),
                    stop=(jb == 7),
                )

        # software pipeline: emit av one j-block behind sim so the in-order
        # PE queue never stalls on the exp of the current block
        prev = None
        for jb in range(8):
            pss = ppm.tile([128, S], f32, tag="mm", name=f"pss{h}_{jb}")
            for nn in range(2):
                if FP8:
                    nc.tensor.matmul(
                        pss[:, nn * 512 : (nn + 1) * 512],
                        lhsT=(k8[b1 : b1 + 32, t, :, jb * 128 : (jb + 1) * 128]),
                        rhs=(q8[b1 : b1 + 32, t, :, nn * 512 : (nn + 1) * 512]),
                        start=True,
                        stop=True,
                        perf_mode=DR,
                    )
                else:
                    nc.tensor.matmul(
                        pss[:, nn * 512 : (nn + 1) * 512],
                        lhsT=(sb_kb[b0 : b0 + 64, t, jb * 128 : (jb + 1) * 128]),
                        rhs=(sb_qb[b0 : b0 + 64, t, nn * 512 : (nn + 1) * 512]),
                        start=True,
                        stop=True,
                    )
            if prev is not None:
                av(*prev)
            et = wk.tile([128, S], bf16, tag="et", name=f"et{h}_{jb}", bufs=3)
            nc.scalar.activation(
                out=et[:, :], in_=pss[:, :], func=AF.Exp,
                scale=(SCALE / 65536.0 if FP8 else SCALE),
            )
            prev = (jb, et)
        av(*prev)
        # park the unnormalized output + denominator row; frees the PSUM buf
        # without waiting for the reciprocal roundtrip.  For the last head the
        # output parks via ACT (idle once the final exp retires), halving the
        # tail's serial DVE chain.
        if h == HEADS - 1:
            nc.scalar.copy(out=sb_or[b0 : b0 + 64, t, :], in_=pso[0:64, :])
        else:
            nc.vector.tensor_copy(out=sb_or[b0 : b0 + 64, t, :], in_=pso[0:64, :])
        dstage = wk.tile([1, S], f32, tag="dstage", name=f"dstage{h}", bufs=2)
        nc.vector.tensor_copy(out=dstage[:, :], in_=pso[64:65, :])
        nc.sync.dma_start(out=den_sh[:, h, :], in_=dstage[:, :])
        with nc.allow_low_precision("softmax denominator reciprocal in bf16"):
            nc.vector.reciprocal(out=rcp_sh[:, h, :], in_=den_sh[:, h, :])

    def attn_norm(h):
        """Broadcast 1/den across partitions via a K=1 PE matmul, then scale.

        Emitted one head late so the PE queue never stalls on the reciprocal
        roundtrip; the broadcast tile borrows a ppm slot (PSUM is full during
        attention).
        """
        t, half = h // 2, h % 2
        b0 = 64 * half
        if NO_BCASTMM:
            # both-SBUF tensor_tensor needs equal base partitions: use a full
            # [128, S] tile and fill rows b0..b0+64 via the doubling chain
            rbc = wk.tile([128, S], bf16, tag="rbc", name=f"rbc{h}", bufs=2)
            nc.sync.dma_start(out=rbc[b0 : b0 + 1, :], in_=rcp_sh[:, h, :])
            w = 1
            while w < 64:
                nc.sync.dma_start(
                    out=rbc[b0 + w : b0 + 2 * w, :], in_=rbc[b0 : b0 + w, :]
                )
                w *= 2
            nc.vector.tensor_mul(
                out=sb_o[b0 : b0 + 64, t, :],
                in0=sb_or[b0 : b0 + 64, t, :],
                in1=rbc[b0 : b0 + 64, :],
            )
            return
        rrow = wk.tile([1, S], bf16, tag="rrow", name=f"rrow{h}", bufs=2)
        nc.sync.dma_start(out=rrow[:, :], in_=rcp_sh[:, h, :])
        pbc = ppo.tile([64, S], f32, tag="ov", name=f"pbc{h}")
        for nn in range(2):
            nc.tensor.matmul(
                pbc[:, nn * 512 : (nn + 1) * 512],
                lhsT=ones_col[0:1, :],
                rhs=rrow[0:1, nn * 512 : (nn + 1) * 512],
                start=True,
                stop=True,
            )
        nc.vector.tensor_mul(
            out=sb_o[b0 : b0 + 64, t, :],
            in0=sb_or[b0 : b0 + 64, t, :],
            in1=pbc[:, :],
        )

    def proj_kk(psf, ob, kk):
        for nn in range(2):
            nc.tensor.matmul(
                psf[:, nn * 512 : (nn + 1) * 512],
                lhsT=(sb_wo[:, kk, ob * 128 : (ob + 1) * 128]),
                rhs=(sb_o[:, kk, nn * 512 : (nn + 1) * 512]),
                start=(kk == 0),
                stop=(kk == 3),
            )

    def proj_out(psf, ob):
        # quarter-granularity bias+store so the output DMAs overlap the
        # remaining ACT work in the epilogue
        for nn in range(2):
            ot = wk.tile([128, 512], f32, tag="ot", name=f"ot{ob}_{nn}", bufs=2)
            nc.scalar.activation(
                out=ot[:, :], in_=psf[:, nn * 512 : (nn + 1) * 512],
                func=AF.Identity, bias=sb_bo[:, ob : ob + 1],
            )
            nc.sync.dma_start(
                out=out_d[ob * 128 : (ob + 1) * 128, nn * 512 : (nn + 1) * 512],
                in_=ot[:, :],
            )

    def proj_t(t):
        """Output-projection partial for attention tile t, accumulated into
        SBUF f32 so the matmuls overlap attention instead of serializing at
        the tail.  t=0 evacuates via ACT with the bias fused; later t's add
        the PSUM partial on DVE; t=3 streams the result out."""
        for ob in range(2):
            psf = ppm.tile([128, S], f32, tag="mm", name=f"psf{ob}_{t}")
            for nn in range(2):
                nc.tensor.matmul(
                    psf[:, nn * 512 : (nn + 1) * 512],
                    lhsT=(sb_wo[:, t, ob * 128 : (ob + 1) * 128]),
                    rhs=(sb_o[:, t, nn * 512 : (nn + 1) * 512]),
                    start=True,
                    stop=True,
                )
            if t == 0:
                nc.scalar.activation(
                    out=acc[:, ob, :], in_=psf[:, :], func=AF.Identity,
                    bias=sb_bo[:, ob : ob + 1],
                )
            else:
                nc.vector.tensor_add(
                    out=acc[:, ob, :], in0=psf[:, :], in1=acc[:, ob, :]
                )
                if t == 3:
                    nc.sync.dma_start(
                        out=out_d[ob * 128 : (ob + 1) * 128, :], in_=acc[:, ob, :]
                    )

    # ---- emission order == scheduling priority ----
    qk_block(0)
    qk_block(4)
    rope_norm(0)
    for jb in range(8):
        v_block(jb)
    qk_block(1)
    qk_block(5)
    rope_norm(1)
    qk_block(2)
    qk_block(6)
    attn_head(0)
    rope_norm(2)
    attn_head(1)
    attn_norm(0)
    qk_block(3)
    qk_block(7)
    attn_head(2)
    attn_norm(1)
    rope_norm(3)
    attn_head(3)
    attn_norm(2)
    if TAILPROJ:
        attn_head(4)
        attn_norm(3)
        attn_head(5)
        attn_norm(4)
        attn_head(6)
        attn_norm(5)
        attn_head(7)
        # projection partials for the ready tiles run BEFORE the last two
        # normalize chains in the in-order PE queue; only kk=3 remains gated
        psf0 = ppm.tile([128, S], f32, tag="mm", name="psf0")
        psf1 = ppm.tile([128, S], f32, tag="mm", name="psf1")
        for kk in range(3):
            proj_kk(psf0, 0, kk)
            proj_kk(psf1, 1, kk)
        attn_norm(6)
        attn_norm(7)
        proj_kk(psf0, 0, 3)
        proj_kk(psf1, 1, 3)
        proj_out(psf0, 0)
        proj_out(psf1, 1)
    else:
        attn_head(4)
        attn_norm(3)
        attn_head(5)
        attn_norm(4)
        proj_t(0)
        attn_head(6)
        attn_norm(5)
        proj_t(1)
        attn_head(7)
        attn_norm(6)
        proj_t(2)
        attn_norm(7)
        proj_t(3)


def _patch_act_tables():
    """Steer the act-table-load pass to one set covering Exp+Ln+Copy.

    The default pass picks the first table set containing each activation
    function, which ping-pongs between exp_and_others and natural_log
    (~2.7us per reload).  Emptying every other set forces all activations
    onto natural_log_exp_and_others -> exactly one load.
    """
    import concourse.bacc as bacc

    if getattr(bacc, "_act_tables_patched", False):
        return
    import concourse.hw_specs as hw_specs

    orig = hw_specs.get_activation_tables

    def patched(arch):
        tables = orig(arch)
        keep = "natural_log_exp_and_others"
        assert keep in tables
        return {
            name: (fns if name == keep else set()) for name, fns in tables.items()
        }

    bacc.get_activation_tables = patched
    bacc._act_tables_patched = True


def _build():
    from contextlib import ExitStack

    import concourse.bacc as bacc
    import concourse.tile as tile
    from concourse import mybir

    _patch_act_tables()

    f32 = mybir.dt.float32
    bf16 = mybir.dt.bfloat16
    f8 = mybir.dt.float8e4
    nc = bacc.Bacc("TRN2", target_bir_lowering=False, debug=False, num_devices=N_CORES)
    aps = (
        nc.dram_tensor("x", [C, S], bf16, kind="ExternalInput").ap(),
        nc.dram_tensor("wqkT", [C, 2 * HID], bf16, kind="ExternalInput").ap(),
        nc.dram_tensor("x8", [128, 2, S], f8, kind="ExternalInput").ap(),
        nc.dram_tensor("wqk8", [128, 2, 2 * HID], f8, kind="ExternalInput").ap(),
        nc.dram_tensor("wvT", [C, HID], bf16, kind="ExternalInput").ap(),
        nc.dram_tensor("woT", [HID, C], bf16, kind="ExternalInput").ap(),
        nc.dram_tensor("bout", [128, 2], f32, kind="ExternalInput").ap(),
        nc.dram_tensor("cosT", [128, S], bf16, kind="ExternalInput").ap(),
        nc.dram_tensor("sinT", [128, S], bf16, kind="ExternalInput").ap(),
        nc.dram_tensor("out", [C, S], f32, kind="ExternalOutput").ap(),
    )
    with tile.TileContext(nc) as tc:
        with ExitStack() as ctx:
            _emit(ctx, tc, aps)
    nc.compile()
    return nc


def _get_nc():
    if "nc" not in _CACHE:
        _CACHE["nc"] = _build()
    return _CACHE["nc"]


def _make_in_maps(x, w_qkv, w_out, b_out):
    import ml_dtypes

    bf = ml_dtypes.bfloat16
    f8 = ml_dtypes.float8_e4m3
    xf32 = np.asarray(x, np.float32).reshape(N_CORES, C, S)
    xf = np.ascontiguousarray(xf32).astype(bf)
    wq = np.asarray(w_qkv, np.float32)
    wqkT = np.ascontiguousarray(wq[0 : 2 * HID, :].T).astype(bf)
    wvT = np.ascontiguousarray(wq[2 * HID : 3 * HID, :].T).astype(bf)
    x8 = np.ascontiguousarray(
        np.stack([xf32[:, 0:128, :], xf32[:, 128:256, :]], axis=2)
    ).astype(f8)
    wqkT256 = wq[0 : 2 * HID, :].T * 256.0
    wqk8 = np.ascontiguousarray(
        np.stack([wqkT256[0:128, :], wqkT256[128:256, :]], axis=1)
    ).astype(f8)
    woT = np.ascontiguousarray(np.asarray(w_out, np.float32).T).astype(bf)
    bo = np.ascontiguousarray(np.asarray(b_out, np.float32).reshape(2, 128).T)
    cosT, sinT = _rope_tables()
    shared = {
        "wqkT": wqkT,
        "wqk8": wqk8,
        "wvT": wvT,
        "woT": woT,
        "bout": bo,
        "cosT": cosT.astype(bf),
        "sinT": sinT.astype(bf),
    }
    return [
        dict(shared, x=np.ascontiguousarray(xf[i]), x8=np.ascontiguousarray(x8[i]))
        for i in range(N_CORES)
    ]


def _postprocess(res):
    out = np.stack([r["out"] for r in res.results], axis=0)
    return out.reshape(N_CORES, C, 32, 32).astype(np.float32)


def _run(x, w_qkv, w_out, b_out, trace=False):
    from concourse.bass_utils import run_bass_kernel_spmd

    nc = _get_nc()
    in_maps = _make_in_maps(x, w_qkv, w_out, b_out)
    res = run_bass_kernel_spmd(nc, in_maps, core_ids=list(range(N_CORES)), trace=trace)
    return _postprocess(res), res


def kernel(x, w_qkv, w_out, b_out):
    return _run(x, w_qkv, w_out, b_out, trace=False)[0]


# revision 76
# speedup vs baseline: 1.6731x; 1.0030x over previous
"""Trainium2 Bass kernel for AttentionWithRotaryPosEmb (8 cores, data-parallel).

Strategy
--------
Data-parallel over batch: each of the 8 NeuronCores computes one batch element
end-to-end. No collectives needed.

Per-core pipeline (batch x_b is [C=256, S=1024]):
  1. QKV projection: q,k as [o, s] (o = 64h+d), vT as [s, hid] straight out of
     the matmuls -- no transposes needed on-chip. PSUM evacuated by ACT copy
     (f32 -> bf16) so DVE stays free and later elementwise runs in bf16 DVE
     perf modes.
  2. RoPE on q,k in [d, s] layout: rotate-half is a single DVE stream_shuffle
     (32-partition-group permutation), then sin/cos multiplies + add against
     host-precomputed row tables (sign of sin encodes rotate-half; rows with
     d >= 32 have sin=0, cos=1).
  3. L2 norm over the sequence axis: square+reduce (ACT for tile 0 on the
     attention warm-up path, DVE after), rsqrt via exp(-0.5*ln(x)) on ACT --
     the activation-table pass is patched so Exp/Ln/Copy share one table set
     with the softmax exp (no table reloads). Both q,k scales fold into q.
  4. Attention with transposed softmax: simT[j, i] via matmul(lhsT=k_h,
     rhs=q_h); exp on ACT with scale=10 fused; softmax denominators come free
     from a ones-column appended to vT (65th lhsT column); outT[d, i]
     accumulates over j-blocks in PSUM.
  5. Softmax normalization: the unnormalized output parks in SBUF immediately
     (frees the PSUM ring); the denominator row is DMA-reshaped to [128, 8]
     so the DVE reciprocal runs multi-partition (0.3us instead of 6.5us
     single-lane), DMA'd back to a row, broadcast across 64 partitions by a
     K=1 PE matmul against a ones column (into a PSUM tile in the ppo pool --
     keeping it out of the ppm ring is critical, in-order PE queues otherwise
     stall the sim pipeline on the reciprocal roundtrip), then one DVE mul.
  6. Output projection accumulated per attention tile in PSUM: the kk=0..2
     partials run while the last two heads' normalize chains drain, only kk=3
     trails them; bias via ACT (per-partition bias AP) and quarter-granular
     stores overlap the epilogue.

  Notes: fp8/DoubleRow paths (K_FP8=1) are implemented but measurably SLOWER
  on this part -- the PE power governor throttles DoubleRow harder than the
  cycle savings.  tensor_tensor_reduce crashes at runtime on this hw/runtime
  combo; GPSIMD extended ops (partition_broadcast, swdge DMA) are unavailable
  (bedrock image, no HIPI ucode).  Exec time is throttle-lottery dependent:
  ~143-190us (median ambient ~150-170us) vs the 238us session baseline.
"""

import os
import sys

import numpy as np

if "/opt/trn_rl_repo" not in sys.path:
    sys.path.insert(0, "/opt/trn_rl_repo")

# bisect fallbacks (baseline-proven implementations).  tensor_tensor_reduce
# crashes at runtime on this hw/runtime combo -> always use mul+reduce_sum.
NO_SHUFFLE = os.environ.get("K_NO_SHUFFLE") == "1"
NO_TTR = True
NO_IDBIAS = os.environ.get("K_NO_IDBIAS") == "1"
NO_BCASTMM = os.environ.get("K_NO_BCASTMM") == "1"
TAILPROJ = os.environ.get("K_TAILPROJ", "1") == "1"
FP8 = os.environ.get("K_FP8") == "1"

HEADS = 8
DH = 64
S = 1024
C = 256
HID = 512
ROT = 32
HALF = 16
SCALE = 10.0
N_CORES = 8

# rotate-half as a 32-partition-group shuffle: swap the 16-row halves
SHUF_MASK = list(range(16, 32)) + list(range(16))

_CACHE = {}


def _rope_tables():
    """Row-patterned cos/sin tables [128, S] matching the q/k SBUF layout.

    Partition row r holds o-row (128t + r) of q/k tile t, i.e. head 2t + (r>=64)
    and d = r % 64.  Rows d in [0,16) get cos(i*invf[d]) / -sin(i*invf[d]);
    rows d in [16,32) get cos(i*invf[d-16]) / +sin(i*invf[d-16]); rows d >= 32
    get cos=1, sin=0 (identity).  The sign of sin encodes rotate_half.
    """
    inv = (
        1.0 / (np.float32(10000.0) ** (np.arange(0, ROT, 2, dtype=np.float32) / np.float32(ROT)))
    ).astype(np.float32)
    ang = (np.arange(S, dtype=np.float32)[None, :] * inv[:, None]).astype(np.float32)
    cos16 = np.cos(ang).astype(np.float32)  # [16, S]
    sin16 = np.sin(ang).astype(np.float32)
    cosT = np.ones((128, S), np.float32)
    sinT = np.zeros((128, S), np.float32)
    for r in range(128):
        d = r % 64
        if d < HALF:
            cosT[r] = cos16[d]
            sinT[r] = -sin16[d]
        elif d < ROT:
            cosT[r] = cos16[d - HALF]
            sinT[r] = sin16[d - HALF]
    return cosT, sinT


def _emit(ctx, tc, aps):
    import concourse.bass as bass  # noqa: F401
    from concourse import mybir

    f32 = mybir.dt.float32
    bf16 = mybir.dt.bfloat16
    f8 = mybir.dt.float8e4
    DR = mybir.MatmulPerfMode.DoubleRow
    AF = mybir.ActivationFunctionType
    ALU = mybir.AluOpType
    nc = tc.nc
    x_d, wqk_d, x8_d, wqk8_d, wv_d, wo_d, bo_d, cos_d, sin_d, out_d = aps

    singles = ctx.enter_context(tc.tile_pool(name="singles", bufs=1))
    wk = ctx.enter_context(tc.tile_pool(name="wk", bufs=3))
    ppm = ctx.enter_context(tc.tile_pool(name="ppm", bufs=2, space="PSUM"))
    ppo = ctx.enter_context(tc.tile_pool(name="ppo", bufs=2, space="PSUM"))

    # ---- persistent SBUF ----
    sb_x = singles.tile([128, 2, S], bf16)
    sb_wv = singles.tile([128, 2, HID], bf16)
    if FP8:
        sb_x8 = singles.tile([128, 2, S], f8)
        sb_wqk8 = singles.tile([128, 2, 2 * HID], f8)
        q8a = singles.tile([128, 4, S], f8)
        k8a = singles.tile([128, 4, S], f8)
        q8 = singles.tile([64, 4, 2, S], f8)
        k8 = singles.tile([64, 4, 2, S], f8)
        ln256 = singles.tile([128, 1], f32)
    else:
        sb_wqk = singles.tile([128, 2, 2 * HID], bf16)
    sb_wo = singles.tile([128, 4, C], bf16)
    sb_bo = singles.tile([128, 2], f32)
    sb_cos = singles.tile([128, S], bf16)
    sb_sin = singles.tile([128, S], bf16)
    sb_q = singles.tile([128, 4, S], bf16)   # raw q -> roped q (pre-scale)
    sb_k = singles.tile([128, 4, S], bf16)   # raw k (k*cos staging in-place)
    sb_qb = singles.tile([128, 4, S], bf16)  # normalized q, sim operand
    sb_kb = singles.tile([128, 4, S], bf16)  # roped k, sim operand
    sb_vx = singles.tile([128, 8, HEADS * (DH + 1)], bf16)
    sb_or = singles.tile([128, 4, S], bf16)  # attention out, unnormalized
    sb_o = singles.tile([128, 4, S], bf16)   # attention out, proj operand
    sb_stat = singles.tile([128, 16], f32)
    den_sh = singles.tile([128, 8, 8], f32)     # partition-spread denominators
    rcp_sh = singles.tile([128, 8, 8], bf16)    # reciprocals (bf16: bcast operand)
    ones_col = singles.tile([1, 64], bf16)      # lhsT for the rcp row broadcast
    acc = singles.tile([128, 2, S], f32)        # output-projection accumulator

    # ---- input loads (c-block granularity so QKV matmuls can start early) ----
    if FP8:
        nc.sync.dma_start(out=sb_wqk8[:, :, :], in_=wqk8_d[:, :])
        nc.sync.dma_start(out=sb_x8[:, :, :], in_=x8_d[:, :])
    else:
        nc.sync.dma_start(out=sb_wqk[:, 0, :], in_=wqk_d[0:128, :])
        nc.sync.dma_start(out=sb_wqk[:, 1, :], in_=wqk_d[128:256, :])
    nc.sync.dma_start(out=sb_x[:, 0, :], in_=x_d[0:128, :])
    nc.sync.dma_start(out=sb_x[:, 1, :], in_=x_d[128:256, :])
    nc.sync.dma_start(out=sb_wv[:, 0, :], in_=wv_d[0:128, :])
    nc.sync.dma_start(out=sb_wv[:, 1, :], in_=wv_d[128:256, :])
    nc.sync.dma_start(out=sb_cos[:, :], in_=cos_d[:, :])
    nc.sync.dma_start(out=sb_sin[:, :], in_=sin_d[:, :])
    for kk in range(4):
        nc.sync.dma_start(out=sb_wo[:, kk, :], in_=wo_d[kk * 128 : (kk + 1) * 128, :])
    nc.sync.dma_start(out=sb_bo[:, :], in_=bo_d[:, :])

    # ones column per head in vT_ext -> free softmax denominators
    vx4 = sb_vx.rearrange("p j (h e) -> p j h e", e=DH + 1)
    nc.vector.memset(vx4[:, :, :, DH : DH + 1], 1.0)
    nc.vector.memset(ones_col[:, :], 1.0)
    if FP8:
        nc.vector.memset(ln256[:, :], float(np.log(256.0)))

    def qk_block(ob):
        """q (ob<4) / k (ob>=4) projection o-block -> SBUF bf16 via ACT."""
        ps = ppm.tile([128, S], f32, tag="mm", name=f"ps_qk{ob}")
        for nn in range(2):
            if FP8:
                nc.tensor.matmul(
                    ps[:, nn * 512 : (nn + 1) * 512],
                    lhsT=(sb_wqk8[:, :, ob * 128 : (ob + 1) * 128]),
                    rhs=(sb_x8[:, :, nn * 512 : (nn + 1) * 512]),
                    start=True,
                    stop=True,
                    perf_mode=DR,
                )
                continue
            for kk in range(2):
                nc.tensor.matmul(
                    ps[:, nn * 512 : (nn + 1) * 512],
                    lhsT=(sb_wqk[:, kk, ob * 128 : (ob + 1) * 128]),
                    rhs=(sb_x[:, kk, nn * 512 : (nn + 1) * 512]),
                    start=(kk == 0),
                    stop=(kk == 1),
                )
        dst3 = sb_q if ob < 4 else sb_k
        nc.scalar.copy(out=dst3[:, ob % 4, :], in_=ps[:, :])

    def rope_norm(t):
        """RoPE + l2-norm stats for q/k tile t; fold both scales into q."""
        for src3, col in ((sb_q, 0), (sb_k, 4)):
            cur = src3[:, t, :]
            rot = wk.tile([128, S], bf16, tag="rot", name=f"rot{t}_{col}", bufs=2)
            # rotate-half: swap 16-row halves of each 32-partition group
            if NO_SHUFFLE:
                for base in range(0, 128, 32):
                    nc.sync.dma_start(
                        out=rot[base : base + 16, :], in_=src3[base + 16 : base + 32, t, :]
                    )
                    nc.sync.dma_start(
                        out=rot[base + 16 : base + 32, :], in_=src3[base : base + 16, t, :]
                    )
            else:
                nc.vector.stream_shuffle(out=rot[:, :], in_=cur, mask=SHUF_MASK)
            nc.vector.tensor_mul(out=rot[:, :], in0=rot[:, :], in1=sb_sin[:, :])
            nc.vector.tensor_mul(out=cur, in0=cur, in1=sb_cos[:, :])
            adddst = cur if col == 0 else sb_kb[:, t, :]
            nc.vector.tensor_add(out=adddst, in0=cur, in1=rot[:, :])
            # fused square + sum over s -> stat col.  Tile 0 is the attention
            # warm-up critical path: run it on ACT (idle then) to shorten the
            # DVE chain; ACT is exp-saturated for later tiles.
            sq = wk.tile([128, S], bf16, tag="sq", name=f"sq{t}_{col}", bufs=2)
            if t == 0:
                nc.scalar.activation(
                    out=sq[:, :], in_=adddst, func=AF.Square,
                    accum_out=sb_stat[:, col + t : col + t + 1],
                )
            elif NO_TTR:
                nc.vector.tensor_mul(out=sq[:, :], in0=adddst, in1=adddst)
                nc.vector.reduce_sum(
                    out=sb_stat[:, col + t : col + t + 1], in_=sq[:, :],
                    axis=mybir.AxisListType.X,
                )
            else:
                nc.vector.tensor_tensor_reduce(
                    out=sq[:, :], in0=adddst, in1=adddst, scale=1.0, scalar=0.0,
                    op0=ALU.mult, op1=ALU.add,
                    accum_out=sb_stat[:, col + t : col + t + 1],
                )
        # rs_comb = (ssq_q * ssq_k)^-1/2 = exp(-0.5*(ln q + ln k)); the x256
        # from the fp8 qk weights cancels (sim is scale-invariant through it)
        nc.scalar.activation(
            out=sb_stat[:, 8 + t : 9 + t], in_=sb_stat[:, t : t + 1],
            func=AF.Ln, bias=0.0,
        )
        nc.scalar.activation(
            out=sb_stat[:, 12 + t : 13 + t], in_=sb_stat[:, 4 + t : 5 + t],
            func=AF.Ln, bias=0.0,
        )
        nc.vector.tensor_add(
            out=sb_stat[:, 8 + t : 9 + t],
            in0=sb_stat[:, 8 + t : 9 + t],
            in1=sb_stat[:, 12 + t : 13 + t],
        )
        if FP8:
            nc.scalar.activation(
                out=sb_stat[:, 12 + t : 13 + t], in_=sb_stat[:, 8 + t : 9 + t],
                func=AF.Exp, scale=-0.25, bias=ln256[:, 0:1],
            )
            sc_col = sb_stat[:, 12 + t : 13 + t]
            nc.vector.tensor_scalar_mul(out=q8a[:, t, :], in0=sb_q[:, t, :], scalar1=sc_col)
            nc.vector.tensor_scalar_mul(out=k8a[:, t, :], in0=sb_kb[:, t, :], scalar1=sc_col)
            for src8, dst8 in ((q8a, q8), (k8a, k8)):
                for hh in range(2):
                    for i in range(2):
                        nc.sync.dma_start(
                            out=dst8[32 * hh : 32 * hh + 32, t, i, :],
                            in_=src8[64 * hh + 32 * i : 64 * hh + 32 * i + 32, t, :],
                        )
        else:
            nc.scalar.activation(
                out=sb_stat[:, 12 + t : 13 + t], in_=sb_stat[:, 8 + t : 9 + t],
                func=AF.Exp, scale=-0.5,
            )
            nc.vector.tensor_scalar_mul(
                out=sb_qb[:, t, :], in0=sb_q[:, t, :],
                scalar1=sb_stat[:, 12 + t : 13 + t],
            )

    def v_block(jb):
        """vT s-block: [s_jb, hid] straight from matmul, strided into vx ext."""
        psv = ppo.tile([128, 512], f32, tag="ov", name=f"psv{jb}")
        for kk in range(2):
            nc.tensor.matmul(
                psv[:, :],
                lhsT=(sb_x[:, kk, jb * 128 : (jb + 1) * 128]),
                rhs=(sb_wv[:, kk, :]),
                start=(kk == 0),
                stop=(kk == 1),
            )
        dst = sb_vx[:, jb, :].rearrange("p (h e) -> p h e", e=DH + 1)[:, :, 0:DH]
        src = psv.rearrange("p (h d) -> p h d", d=DH)
        nc.vector.tensor_copy(out=dst, in_=src)  # f32 PSUM -> bf16 SBUF cast

    def attn_head(h):
        t, half = h // 2, h % 2
        b0 = 64 * half
        b1 = 32 * half
        pso = ppo.tile([65, S], f32, tag="ov", name=f"pso{h}")

        def av(jb, et):
            for nn in range(2):
                nc.tensor.matmul(
                    pso[0:65, nn * 512 : (nn + 1) * 512],
                    lhsT=(sb_vx[:, jb, h * (DH + 1) : (h + 1) * (DH + 1)]),
                    rhs=(et[:, nn * 512 : (nn + 1) * 512]),
                    start=(jb == 0),
                    stop=(jb == 7),
                )

        # software pipeline: emit av one j-block behind sim so the in-order
        # PE queue never stalls on the exp of the current block
        prev = None
        for jb in range(8):
            pss = ppm.tile([128, S], f32, tag="mm", name=f"pss{h}_{jb}")
            for nn in range(2):
                if FP8:
                    nc.tensor.matmul(
                        pss[:, nn * 512 : (nn + 1) * 512],
                        lhsT=(k8[b1 : b1 + 32, t, :, jb * 128 : (jb + 1) * 128]),
                        rhs=(q8[b1 : b1 + 32, t, :, nn * 512 : (nn + 1) * 512]),
                        start=True,
                        stop=True,
                        perf_mode=DR,
                    )
                else:
                    nc.tensor.matmul(
                        pss[:, nn * 512 : (nn + 1) * 512],
                        lhsT=(sb_kb[b0 : b0 + 64, t, jb * 128 : (jb + 1) * 128]),
                        rhs=(sb_qb[b0 : b0 + 64, t, nn * 512 : (nn + 1) * 512]),
                        start=True,
                        stop=True,
                    )
            if prev is not None:
                av(*prev)
            et = wk.tile([128, S], bf16, tag="et", name=f"et{h}_{jb}", bufs=3)
            nc.scalar.activation(
                out=et[:, :], in_=pss[:, :], func=AF.Exp,
                scale=(SCALE / 65536.0 if FP8 else SCALE),
            )
            prev = (jb, et)
        av(*prev)
        # park the unnormalized output + denominator row; frees the PSUM buf
        # without waiting for the reciprocal roundtrip.  For the last head the
        # output parks via ACT (idle once the final exp retires), halving the
        # tail's serial DVE chain.
        if h == HEADS - 1:
            nc.scalar.copy(out=sb_or[b0 : b0 + 64, t, :], in_=pso[0:64, :])
        else:
            nc.vector.tensor_copy(out=sb_or[b0 : b0 + 64, t, :], in_=pso[0:64, :])
        dstage = wk.tile([1, S], f32, tag="dstage", name=f"dstage{h}", bufs=2)
        nc.vector.tensor_copy(out=dstage[:, :], in_=pso[64:65, :])
        nc.sync.dma_start(out=den_sh[:, h, :], in_=dstage[:, :])
        with nc.allow_low_precision("softmax denominator reciprocal in bf16"):
            nc.vector.reciprocal(out=rcp_sh[:, h, :], in_=den_sh[:, h, :])

    def attn_norm(h):
        """Broadcast 1/den across partitions via a K=1 PE matmul, then scale.

        Emitted one head late so the PE queue never stalls on the reciprocal
        roundtrip; the broadcast tile borrows a ppm slot (PSUM is full during
        attention).
        """
        t, half = h // 2, h % 2
        b0 = 64 * half
        if NO_BCASTMM:
            # both-SBUF tensor_tensor needs equal base partitions: use a full
            # [128, S] tile and fill rows b0..b0+64 via the doubling chain
            rbc = wk.tile([128, S], bf16, tag="rbc", name=f"rbc{h}", bufs=2)
            nc.sync.dma_start(out=rbc[b0 : b0 + 1, :], in_=rcp_sh[:, h, :])
            w = 1
            while w < 64:
                nc.sync.dma_start(
                    out=rbc[b0 + w : b0 + 2 * w, :], in_=rbc[b0 : b0 + w, :]
                )
                w *= 2
            nc.vector.tensor_mul(
                out=sb_o[b0 : b0 + 64, t, :],
                in0=sb_or[b0 : b0 + 64, t, :],
                in1=rbc[b0 : b0 + 64, :],
            )
            return
        rrow = wk.tile([1, S], bf16, tag="rrow", name=f"rrow{h}", bufs=2)
        nc.sync.dma_start(out=rrow[:, :], in_=rcp_sh[:, h, :])
        pbc = ppo.tile([64, S], f32, tag="ov", name=f"pbc{h}")
        for nn in range(2):
            nc.tensor.matmul(
                pbc[:, nn * 512 : (nn + 1) * 512],
                lhsT=ones_col[0:1, :],
                rhs=rrow[0:1, nn * 512 : (nn + 1) * 512],
                start=True,
                stop=True,
            )
        nc.vector.tensor_mul(
            out=sb_o[b0 : b0 + 64, t, :],
            in0=sb_or[b0 : b0 + 64, t, :],
            in1=pbc[:, :],
        )

    def proj_kk(psf, ob, kk):
        for nn in range(2):
            nc.tensor.matmul(
                psf[:, nn * 512 : (nn + 1) * 512],
                lhsT=(sb_wo[:, kk, ob * 128 : (ob + 1) * 128]),
                rhs=(sb_o[:, kk, nn * 512 : (nn + 1) * 512]),
                start=(kk == 0),
                stop=(kk == 3),
            )

    def proj_out(psf, ob):
        # quarter-granularity bias+store so the output DMAs overlap the
        # remaining ACT work in the epilogue
        for nn in range(2):
            ot = wk.tile([128, 512], f32, tag="ot", name=f"ot{ob}_{nn}", bufs=2)
            nc.scalar.activation(
                out=ot[:, :], in_=psf[:, nn * 512 : (nn + 1) * 512],
                func=AF.Identity, bias=sb_bo[:, ob : ob + 1],
            )
            nc.sync.dma_start(
                out=out_d[ob * 128 : (ob + 1) * 128, nn * 512 : (nn + 1) * 512],
                in_=ot[:, :],
            )

    def proj_t(t):
        """Output-projection partial for attention tile t, accumulated into
        SBUF f32 so the matmuls overlap attention instead of serializing at
        the tail.  t=0 evacuates via ACT with the bias fused; later t's add
        the PSUM partial on DVE; t=3 streams the result out."""
        for ob in range(2):
            psf = ppm.tile([128, S], f32, tag="mm", name=f"psf{ob}_{t}")
            for nn in range(2):
                nc.tensor.matmul(
                    psf[:, nn * 512 : (nn + 1) * 512],
                    lhsT=(sb_wo[:, t, ob * 128 : (ob + 1) * 128]),
                    rhs=(sb_o[:, t, nn * 512 : (nn + 1) * 512]),
                    start=True,
                    stop=True,
                )
            if t == 0:
                nc.scalar.activation(
                    out=acc[:, ob, :], in_=psf[:, :], func=AF.Identity,
                    bias=sb_bo[:, ob : ob + 1],
                )
            else:
                nc.vector.tensor_add(
                    out=acc[:, ob, :], in0=psf[:, :], in1=acc[:, ob, :]
                )
                if t == 3:
                    nc.sync.dma_start(
                        out=out_d[ob * 128 : (ob + 1) * 128, :], in_=acc[:, ob, :]
                    )

    # ---- emission order == scheduling priority ----
    qk_block(0)
    qk_block(4)
    rope_norm(0)
    for jb in range(8):
        v_block(jb)
    qk_block(1)
    qk_block(5)
    rope_norm(1)
    qk_block(2)
    qk_block(6)
    attn_head(0)
    rope_norm(2)
    attn_head(1)
    attn_norm(0)
    qk_block(3)
    qk_block(7)
    attn_head(2)
    attn_norm(1)
    rope_norm(3)
    attn_head(3)
    attn_norm(2)
    if TAILPROJ:
        attn_head(4)
        attn_norm(3)
        attn_head(5)
        attn_norm(4)
        attn_head(6)
        attn_norm(5)
        attn_head(7)
        # projection partials for the ready tiles run BEFORE the last two
        # normalize chains in the in-order PE queue; only kk=3 remains gated
        psf0 = ppm.tile([128, S], f32, tag="mm", name="psf0")
        psf1 = ppm.tile([128, S], f32, tag="mm", name="psf1")
        for kk in range(3):
            proj_kk(psf0, 0, kk)
            proj_kk(psf1, 1, kk)
        attn_norm(6)
        attn_norm(7)
        proj_kk(psf0, 0, 3)
        proj_kk(psf1, 1, 3)
        proj_out(psf0, 0)
        proj_out(psf1, 1)
    else:
        attn_head(4)
        attn_norm(3)
        attn_head(5)
        attn_norm(4)
        proj_t(0)
        attn_head(6)
        attn_norm(5)
        proj_t(1)
        attn_head(7)
        attn_norm(6)
        proj_t(2)
        attn_norm(7)
        proj_t(3)


def _patch_act_tables():
    """Steer the act-table-load pass to one set covering Exp+Ln+Copy.

    The default pass picks the first table set containing each activation
    function, which ping-pongs between exp_and_others and natural_log
    (~2.7us per reload).  Emptying every other set forces all activations
    onto natural_log_exp_and_others -> exactly one load.
    """
    import concourse.bacc as bacc

    if getattr(bacc, "_act_tables_patched", False):
        return
    import concourse.hw_specs as hw_specs

    orig = hw_specs.get_activation_tables

    def patched(arch):
        tables = orig(arch)
        keep = "natural_log_exp_and_others"
        assert keep in tables
        return {
            name: (fns if name == keep else set()) for name, fns in tables.items()
        }

    bacc.get_activation_tables = patched
    bacc._act_tables_patched = True


def _build():
    from contextlib import ExitStack

    import concourse.bacc as bacc
    import concourse.tile as tile
    from concourse import mybir

    _patch_act_tables()

    f32 = mybir.dt.float32
    bf16 = mybir.dt.bfloat16
    f8 = mybir.dt.float8e4
    nc = bacc.Bacc("TRN2", target_bir_lowering=False, debug=False, num_devices=N_CORES)
    aps = (
        nc.dram_tensor("x", [C, S], bf16, kind="ExternalInput").ap(),
        nc.dram_tensor("wqkT", [C, 2 * HID], bf16, kind="ExternalInput").ap(),
        nc.dram_tensor("x8", [128, 2, S], f8, kind="ExternalInput").ap(),
        nc.dram_tensor("wqk8", [128, 2, 2 * HID], f8, kind="ExternalInput").ap(),
        nc.dram_tensor("wvT", [C, HID], bf16, kind="ExternalInput").ap(),
        nc.dram_tensor("woT", [HID, C], bf16, kind="ExternalInput").ap(),
        nc.dram_tensor("bout", [128, 2], f32, kind="ExternalInput").ap(),
        nc.dram_tensor("cosT", [128, S], bf16, kind="ExternalInput").ap(),
        nc.dram_tensor("sinT", [128, S], bf16, kind="ExternalInput").ap(),
        nc.dram_tensor("out", [C, S], f32, kind="ExternalOutput").ap(),
    )
    with tile.TileContext(nc) as tc:
        with ExitStack() as ctx:
            _emit(ctx, tc, aps)
    nc.compile()
    return nc


def _get_nc():
    if "nc" not in _CACHE:
        _CACHE["nc"] = _build()
    return _CACHE["nc"]


def _make_in_maps(x, w_qkv, w_out, b_out):
    import ml_dtypes

    bf = ml_dtypes.bfloat16
    f8 = ml_dtypes.float8_e4m3
    xf32 = np.asarray(x, np.float32).reshape(N_CORES, C, S)
    xf = np.ascontiguousarray(xf32).astype(bf)
    wq = np.asarray(w_qkv, np.float32)
    wqkT = np.ascontiguousarray(wq[0 : 2 * HID, :].T).astype(bf)
    wvT = np.ascontiguousarray(wq[2 * HID : 3 * HID, :].T).astype(bf)
    x8 = np.ascontiguousarray(
        np.stack([xf32[:, 0:128, :], xf32[:, 128:256, :]], axis=2)
    ).astype(f8)
    wqkT256 = wq[0 : 2 * HID, :].T * 256.0
    wqk8 = np.ascontiguousarray(
        np.stack([wqkT256[0:128, :], wqkT256[128:256, :]], axis=1)
    ).astype(f8)
    woT = np.ascontiguousarray(np.asarray(w_out, np.float32).T).astype(bf)
    bo = np.ascontiguousarray(np.asarray(b_out, np.float32).reshape(2, 128).T)
    cosT, sinT = _rope_tables()
    shared = {
        "wqkT": wqkT,
        "wqk8": wqk8,
        "wvT": wvT,
        "woT": woT,
        "bout": bo,
        "cosT": cosT.astype(bf),
        "sinT": sinT.astype(bf),
    }
    return [
        dict(shared, x=np.ascontiguousarray(xf[i]), x8=np.ascontiguousarray(x8[i]))
        for i in range(N_CORES)
    ]


def _postprocess(res):
    out = np.stack([r["out"] for r in res.results], axis=0)
    return out.reshape(N_CORES, C, 32, 32).astype(np.float32)


def _run(x, w_qkv, w_out, b_out, trace=False):
    from concourse.bass_utils import run_bass_kernel_spmd

    nc = _get_nc()
    in_maps = _make_in_maps(x, w_qkv, w_out, b_out)
    res = run_bass_kernel_spmd(nc, in_maps, core_ids=list(range(N_CORES)), trace=trace)
    return _postprocess(res), res


def kernel(x, w_qkv, w_out, b_out):
    return _run(x, w_qkv, w_out, b_out, trace=False)[0]


# revision 77
# speedup vs baseline: 1.6831x; 1.0060x over previous
"""Trainium2 Bass kernel for AttentionWithRotaryPosEmb (8 cores, data-parallel).

Strategy
--------
Data-parallel over batch: each of the 8 NeuronCores computes one batch element
end-to-end. No collectives needed.

Per-core pipeline (batch x_b is [C=256, S=1024]):
  1. QKV projection: q,k as [o, s] (o = 64h+d), vT as [s, hid] straight out of
     the matmuls -- no transposes needed on-chip. PSUM evacuated by ACT copy
     (f32 -> bf16) so DVE stays free and later elementwise runs in bf16 DVE
     perf modes.
  2. RoPE on q,k in [d, s] layout: rotate-half is a single DVE stream_shuffle
     (32-partition-group permutation), then sin/cos multiplies + add against
     host-precomputed row tables (sign of sin encodes rotate-half; rows with
     d >= 32 have sin=0, cos=1).
  3. L2 norm over the sequence axis: square+reduce (ACT for tile 0 on the
     attention warm-up path, DVE after), rsqrt via exp(-0.5*ln(x)) on ACT --
     the activation-table pass is patched so Exp/Ln/Copy share one table set
     with the softmax exp (no table reloads). Both q,k scales fold into q.
  4. Attention with transposed softmax: simT[j, i] via matmul(lhsT=k_h,
     rhs=q_h); exp on ACT with scale=10 fused; softmax denominators come free
     from a ones-column appended to vT (65th lhsT column); outT[d, i]
     accumulates over j-blocks in PSUM.
  5. Softmax normalization: the unnormalized output parks in SBUF immediately
     (frees the PSUM ring); the denominator row is DMA-reshaped to [128, 8]
     so the DVE reciprocal runs multi-partition (0.3us instead of 6.5us
     single-lane), DMA'd back to a row, broadcast across 64 partitions by a
     K=1 PE matmul against a ones column (into a PSUM tile in the ppo pool --
     keeping it out of the ppm ring is critical, in-order PE queues otherwise
     stall the sim pipeline on the reciprocal roundtrip), then one DVE mul.
  6. Output projection accumulated per attention tile in PSUM: the kk=0..2
     partials run while the last two heads' normalize chains drain, only kk=3
     trails them; bias via ACT (per-partition bias AP) and quarter-granular
     stores overlap the epilogue.

  Notes: fp8/DoubleRow paths (K_FP8=1) are implemented but measurably SLOWER
  on this part -- the PE power governor throttles DoubleRow harder than the
  cycle savings.  tensor_tensor_reduce crashes at runtime on this hw/runtime
  combo; GPSIMD extended ops (partition_broadcast, swdge DMA) are unavailable
  (bedrock image, no HIPI ucode).  Exec time is throttle-lottery dependent:
  ~143-190us (median ambient ~150-170us) vs the 238us session baseline.
"""

import os
import sys

import numpy as np

if "/opt/trn_rl_repo" not in sys.path:
    sys.path.insert(0, "/opt/trn_rl_repo")

# bisect fallbacks (baseline-proven implementations).  tensor_tensor_reduce
# crashes at runtime on this hw/runtime combo -> always use mul+reduce_sum.
NO_SHUFFLE = os.environ.get("K_NO_SHUFFLE") == "1"
NO_TTR = True
NO_IDBIAS = os.environ.get("K_NO_IDBIAS") == "1"
NO_BCASTMM = os.environ.get("K_NO_BCASTMM") == "1"
TAILPROJ = os.environ.get("K_TAILPROJ", "1") == "1"
FP8 = os.environ.get("K_FP8") == "1"

HEADS = 8
DH = 64
S = 1024
C = 256
HID = 512
ROT = 32
HALF = 16
SCALE = 10.0
N_CORES = 8

# rotate-half as a 32-partition-group shuffle: swap the 16-row halves
SHUF_MASK = list(range(16, 32)) + list(range(16))

_CACHE = {}


def _rope_tables():
    """Row-patterned cos/sin tables [128, S] matching the q/k SBUF layout.

    Partition row r holds o-row (128t + r) of q/k tile t, i.e. head 2t + (r>=64)
    and d = r % 64.  Rows d in [0,16) get cos(i*invf[d]) / -sin(i*invf[d]);
    rows d in [16,32) get cos(i*invf[d-16]) / +sin(i*invf[d-16]); rows d >= 32
    get cos=1, sin=0 (identity).  The sign of sin encodes rotate_half.
    """
    inv = (
        1.0 / (np.float32(10000.0) ** (np.arange(0, ROT, 2, dtype=np.float32) / np.float32(ROT)))
    ).astype(np.float32)
    ang = (np.arange(S, dtype=np.float32)[None, :] * inv[:, None]).astype(np.float32)
    cos16 = np.cos(ang).astype(np.float32)  # [16, S]
    sin16 = np.sin(ang).astype(np.float32)
    cosT = np.ones((128, S), np.float32)
    sinT = np.zeros((128, S), np.float32)
    for r in range(128):
        d = r % 64
        if d < HALF:
            cosT[r] = cos16[d]
            sinT[r] = -sin16[d]
        elif d < ROT:
            cosT[r] = cos16[d - HALF]
            sinT[r] = sin16[d - HALF]
    return cosT, sinT


def _emit(ctx, tc, aps):
    import concourse.bass as bass  # noqa: F401
    from concourse import mybir

    f32 = mybir.dt.float32
    bf16 = mybir.dt.bfloat16
    f8 = mybir.dt.float8e4
    DR = mybir.MatmulPerfMode.DoubleRow
    AF = mybir.ActivationFunctionType
    ALU = mybir.AluOpType
    nc = tc.nc
    x_d, wqk_d, x8_d, wqk8_d, wv_d, wo_d, bo_d, cos_d, sin_d, out_d = aps

    singles = ctx.enter_context(tc.tile_pool(name="singles", bufs=1))
    wk = ctx.enter_context(tc.tile_pool(name="wk", bufs=3))
    ppm = ctx.enter_context(tc.tile_pool(name="ppm", bufs=2, space="PSUM"))
    ppo = ctx.enter_context(tc.tile_pool(name="ppo", bufs=2, space="PSUM"))

    # ---- persistent SBUF ----
    sb_x = singles.tile([128, 2, S], bf16)
    sb_wv = singles.tile([128, 2, HID], bf16)
    if FP8:
        sb_x8 = singles.tile([128, 2, S], f8)
        sb_wqk8 = singles.tile([128, 2, 2 * HID], f8)
        q8a = singles.tile([128, 4, S], f8)
        k8a = singles.tile([128, 4, S], f8)
        q8 = singles.tile([64, 4, 2, S], f8)
        k8 = singles.tile([64, 4, 2, S], f8)
        ln256 = singles.tile([128, 1], f32)
    else:
        sb_wqk = singles.tile([128, 2, 2 * HID], bf16)
    sb_wo = singles.tile([128, 4, C], bf16)
    sb_bo = singles.tile([128, 2], f32)
    sb_cos = singles.tile([128, S], bf16)
    sb_sin = singles.tile([128, S], bf16)
    sb_q = singles.tile([128, 4, S], bf16)   # raw q -> roped q (pre-scale)
    sb_k = singles.tile([128, 4, S], bf16)   # raw k (k*cos staging in-place)
    sb_qb = singles.tile([128, 4, S], bf16)  # normalized q, sim operand
    sb_kb = singles.tile([128, 4, S], bf16)  # roped k, sim operand
    sb_vx = singles.tile([128, 8, HEADS * (DH + 1)], bf16)
    sb_or = singles.tile([128, 4, S], bf16)  # attention out, unnormalized
    sb_o = singles.tile([128, 4, S], bf16)   # attention out, proj operand
    sb_stat = singles.tile([128, 16], f32)
    den_sh = singles.tile([128, 8, 8], f32)     # partition-spread denominators
    rcp_sh = singles.tile([128, 8, 8], bf16)    # reciprocals (bf16: bcast operand)
    ones_col = singles.tile([1, 64], bf16)      # lhsT for the rcp row broadcast
    acc = singles.tile([128, 2, S], f32)        # output-projection accumulator

    # ---- input loads (c-block granularity so QKV matmuls can start early) ----
    if FP8:
        nc.sync.dma_start(out=sb_wqk8[:, :, :], in_=wqk8_d[:, :])
        nc.sync.dma_start(out=sb_x8[:, :, :], in_=x8_d[:, :])
    else:
        nc.sync.dma_start(out=sb_wqk[:, 0, :], in_=wqk_d[0:128, :])
        nc.sync.dma_start(out=sb_wqk[:, 1, :], in_=wqk_d[128:256, :])
    nc.sync.dma_start(out=sb_x[:, 0, :], in_=x_d[0:128, :])
    nc.sync.dma_start(out=sb_x[:, 1, :], in_=x_d[128:256, :])
    nc.sync.dma_start(out=sb_wv[:, 0, :], in_=wv_d[0:128, :])
    nc.sync.dma_start(out=sb_wv[:, 1, :], in_=wv_d[128:256, :])
    nc.sync.dma_start(out=sb_cos[:, :], in_=cos_d[:, :])
    nc.sync.dma_start(out=sb_sin[:, :], in_=sin_d[:, :])
    for kk in range(4):
        nc.sync.dma_start(out=sb_wo[:, kk, :], in_=wo_d[kk * 128 : (kk + 1) * 128, :])
    nc.sync.dma_start(out=sb_bo[:, :], in_=bo_d[:, :])

    # ones column per head in vT_ext -> free softmax denominators
    vx4 = sb_vx.rearrange("p j (h e) -> p j h e", e=DH + 1)
    nc.vector.memset(vx4[:, :, :, DH : DH + 1], 1.0)
    nc.vector.memset(ones_col[:, :], 1.0)
    if FP8:
        nc.vector.memset(ln256[:, :], float(np.log(256.0)))

    def qk_block(ob):
        """q (ob<4) / k (ob>=4) projection o-block -> SBUF bf16 via ACT."""
        ps = ppm.tile([128, S], f32, tag="mm", name=f"ps_qk{ob}")
        for nn in range(2):
            if FP8:
                nc.tensor.matmul(
                    ps[:, nn * 512 : (nn + 1) * 512],
                    lhsT=(sb_wqk8[:, :, ob * 128 : (ob + 1) * 128]),
                    rhs=(sb_x8[:, :, nn * 512 : (nn + 1) * 512]),
                    start=True,
                    stop=True,
                    perf_mode=DR,
                )
                continue
            for kk in range(2):
                nc.tensor.matmul(
                    ps[:, nn * 512 : (nn + 1) * 512],
                    lhsT=(sb_wqk[:, kk, ob * 128 : (ob + 1) * 128]),
                    rhs=(sb_x[:, kk, nn * 512 : (nn + 1) * 512]),
                    start=(kk == 0),
                    stop=(kk == 1),
                )
        dst3 = sb_q if ob < 4 else sb_k
        nc.scalar.copy(out=dst3[:, ob % 4, :], in_=ps[:, :])

    def rope_norm(t):
        """RoPE + l2-norm stats for q/k tile t; fold both scales into q."""
        for src3, col in ((sb_q, 0), (sb_k, 4)):
            cur = src3[:, t, :]
            rot = wk.tile([128, S], bf16, tag="rot", name=f"rot{t}_{col}", bufs=2)
            # rotate-half: swap 16-row halves of each 32-partition group
            if NO_SHUFFLE:
                for base in range(0, 128, 32):
                    nc.sync.dma_start(
                        out=rot[base : base + 16, :], in_=src3[base + 16 : base + 32, t, :]
                    )
                    nc.sync.dma_start(
                        out=rot[base + 16 : base + 32, :], in_=src3[base : base + 16, t, :]
                    )
            else:
                nc.vector.stream_shuffle(out=rot[:, :], in_=cur, mask=SHUF_MASK)
            nc.vector.tensor_mul(out=rot[:, :], in0=rot[:, :], in1=sb_sin[:, :])
            nc.vector.tensor_mul(out=cur, in0=cur, in1=sb_cos[:, :])
            adddst = cur if col == 0 else sb_kb[:, t, :]
            nc.vector.tensor_add(out=adddst, in0=cur, in1=rot[:, :])
            # fused square + sum over s -> stat col.  Tile 0 is the attention
            # warm-up critical path: run it on ACT (idle then) to shorten the
            # DVE chain; ACT is exp-saturated for later tiles.
            sq = wk.tile([128, S], bf16, tag="sq", name=f"sq{t}_{col}", bufs=2)
            if t == 0:
                nc.scalar.activation(
                    out=sq[:, :], in_=adddst, func=AF.Square,
                    accum_out=sb_stat[:, col + t : col + t + 1],
                )
            elif NO_TTR:
                nc.vector.tensor_mul(out=sq[:, :], in0=adddst, in1=adddst)
                nc.vector.reduce_sum(
                    out=sb_stat[:, col + t : col + t + 1], in_=sq[:, :],
                    axis=mybir.AxisListType.X,
                )
            else:
                nc.vector.tensor_tensor_reduce(
                    out=sq[:, :], in0=adddst, in1=adddst, scale=1.0, scalar=0.0,
                    op0=ALU.mult, op1=ALU.add,
                    accum_out=sb_stat[:, col + t : col + t + 1],
                )
        # rs_comb = (ssq_q * ssq_k)^-1/2 = exp(-0.5*(ln q + ln k)); the x256
        # from the fp8 qk weights cancels (sim is scale-invariant through it)
        nc.scalar.activation(
            out=sb_stat[:, 8 + t : 9 + t], in_=sb_stat[:, t : t + 1],
            func=AF.Ln, bias=0.0,
        )
        nc.scalar.activation(
            out=sb_stat[:, 12 + t : 13 + t], in_=sb_stat[:, 4 + t : 5 + t],
            func=AF.Ln, bias=0.0,
        )
        nc.vector.tensor_add(
            out=sb_stat[:, 8 + t : 9 + t],
            in0=sb_stat[:, 8 + t : 9 + t],
            in1=sb_stat[:, 12 + t : 13 + t],
        )
        if FP8:
            nc.scalar.activation(
                out=sb_stat[:, 12 + t : 13 + t], in_=sb_stat[:, 8 + t : 9 + t],
                func=AF.Exp, scale=-0.25, bias=ln256[:, 0:1],
            )
            sc_col = sb_stat[:, 12 + t : 13 + t]
            nc.vector.tensor_scalar_mul(out=q8a[:, t, :], in0=sb_q[:, t, :], scalar1=sc_col)
            nc.vector.tensor_scalar_mul(out=k8a[:, t, :], in0=sb_kb[:, t, :], scalar1=sc_col)
            for src8, dst8 in ((q8a, q8), (k8a, k8)):
                for hh in range(2):
                    for i in range(2):
                        nc.sync.dma_start(
                            out=dst8[32 * hh : 32 * hh + 32, t, i, :],
                            in_=src8[64 * hh + 32 * i : 64 * hh + 32 * i + 32, t, :],
                        )
        else:
            nc.scalar.activation(
                out=sb_stat[:, 12 + t : 13 + t], in_=sb_stat[:, 8 + t : 9 + t],
                func=AF.Exp, scale=-0.5,
            )
            nc.vector.tensor_scalar_mul(
                out=sb_qb[:, t, :], in0=sb_q[:, t, :],
                scalar1=sb_stat[:, 12 + t : 13 + t],
            )

    def v_block(jb):
        """vT s-block: [s_jb, hid] straight from matmul, strided into vx ext."""
        psv = ppo.tile([128, 512], f32, tag="ov", name=f"psv{jb}")
        for kk in range(2):
            nc.tensor.matmul(
                psv[:, :],
                lhsT=(sb_x[:, kk, jb * 128 : (jb + 1) * 128]),
                rhs=(sb_wv[:, kk, :]),
                start=(kk == 0),
                stop=(kk == 1),
            )
        dst = sb_vx[:, jb, :].rearrange("p (h e) -> p h e", e=DH + 1)[:, :, 0:DH]
        src = psv.rearrange("p (h d) -> p h d", d=DH)
        nc.vector.tensor_copy(out=dst, in_=src)  # f32 PSUM -> bf16 SBUF cast

    def attn_head(h):
        t, half = h // 2, h % 2
        b0 = 64 * half
        b1 = 32 * half
        pso = ppo.tile([65, S], f32, tag="ov", name=f"pso{h}")

        def av(jb, et):
            for nn in range(2):
                nc.tensor.matmul(
                    pso[0:65, nn * 512 : (nn + 1) * 512],
                    lhsT=(sb_vx[:, jb, h * (DH + 1) : (h + 1) * (DH + 1)]),
                    rhs=(et[:, nn * 512 : (nn + 1) * 512]),
                    start=(jb == 0),
                    stop=(jb == 7),
                )

        # software pipeline: emit av one j-block behind sim so the in-order
        # PE queue never stalls on the exp of the current block
        prev = None
        for jb in range(8):
            pss = ppm.tile([128, S], f32, tag="mm", name=f"pss{h}_{jb}")
            for nn in range(2):
                if FP8:
                    nc.tensor.matmul(
                        pss[:, nn * 512 : (nn + 1) * 512],
                        lhsT=(k8[b1 : b1 + 32, t, :, jb * 128 : (jb + 1) * 128]),
                        rhs=(q8[b1 : b1 + 32, t, :, nn * 512 : (nn + 1) * 512]),
                        start=True,
                        stop=True,
                        perf_mode=DR,
                    )
                else:
                    nc.tensor.matmul(
                        pss[:, nn * 512 : (nn + 1) * 512],
                        lhsT=(sb_kb[b0 : b0 + 64, t, jb * 128 : (jb + 1) * 128]),
                        rhs=(sb_qb[b0 : b0 + 64, t, nn * 512 : (nn + 1) * 512]),
                        start=True,
                        stop=True,
                    )
            if prev is not None:
                av(*prev)
            et = wk.tile([128, S], bf16, tag="et", name=f"et{h}_{jb}", bufs=3)
            nc.scalar.activation(
                out=et[:, :], in_=pss[:, :], func=AF.Exp,
                scale=(SCALE / 65536.0 if FP8 else SCALE),
            )
            prev = (jb, et)
        av(*prev)
        # park the unnormalized output + denominator row; frees the PSUM buf
        # without waiting for the reciprocal roundtrip.  For the last head the
        # output parks via ACT (idle once the final exp retires), halving the
        # tail's serial DVE chain.
        if h == HEADS - 1:
            nc.scalar.copy(out=sb_or[b0 : b0 + 64, t, :], in_=pso[0:64, :])
        else:
            nc.vector.tensor_copy(out=sb_or[b0 : b0 + 64, t, :], in_=pso[0:64, :])
        dstage = wk.tile([1, S], f32, tag="dstage", name=f"dstage{h}", bufs=2)
        nc.vector.tensor_copy(out=dstage[:, :], in_=pso[64:65, :])
        nc.sync.dma_start(out=den_sh[:, h, :], in_=dstage[:, :])
        with nc.allow_low_precision("softmax denominator reciprocal in bf16"):
            nc.vector.reciprocal(out=rcp_sh[:, h, :], in_=den_sh[:, h, :])

    def attn_norm(h):
        """Broadcast 1/den across partitions via a K=1 PE matmul, then scale.

        Emitted one head late so the PE queue never stalls on the reciprocal
        roundtrip; the broadcast tile borrows a ppm slot (PSUM is full during
        attention).
        """
        t, half = h // 2, h % 2
        b0 = 64 * half
        if NO_BCASTMM:
            # both-SBUF tensor_tensor needs equal base partitions: use a full
            # [128, S] tile and fill rows b0..b0+64 via the doubling chain
            rbc = wk.tile([128, S], bf16, tag="rbc", name=f"rbc{h}", bufs=2)
            nc.sync.dma_start(out=rbc[b0 : b0 + 1, :], in_=rcp_sh[:, h, :])
            w = 1
            while w < 64:
                nc.sync.dma_start(
                    out=rbc[b0 + w : b0 + 2 * w, :], in_=rbc[b0 : b0 + w, :]
                )
                w *= 2
            nc.vector.tensor_mul(
                out=sb_o[b0 : b0 + 64, t, :],
                in0=sb_or[b0 : b0 + 64, t, :],
                in1=rbc[b0 : b0 + 64, :],
            )
            return
        rrow = wk.tile([1, S], bf16, tag="rrow", name=f"rrow{h}", bufs=2)
        nc.sync.dma_start(out=rrow[:, :], in_=rcp_sh[:, h, :])
        pbc = ppo.tile([64, S], f32, tag="ov", name=f"pbc{h}")
        for nn in range(2):
            nc.tensor.matmul(
                pbc[:, nn * 512 : (nn + 1) * 512],
                lhsT=ones_col[0:1, :],
                rhs=rrow[0:1, nn * 512 : (nn + 1) * 512],
                start=True,
                stop=True,
            )
        nc.vector.tensor_mul(
            out=sb_o[b0 : b0 + 64, t, :],
            in0=sb_or[b0 : b0 + 64, t, :],
            in1=pbc[:, :],
        )

    def proj_kk(psf, ob, kk):
        for nn in range(2):
            nc.tensor.matmul(
                psf[:, nn * 512 : (nn + 1) * 512],
                lhsT=(sb_wo[:, kk, ob * 128 : (ob + 1) * 128]),
                rhs=(sb_o[:, kk, nn * 512 : (nn + 1) * 512]),
                start=(kk == 0),
                stop=(kk == 3),
            )

    def proj_out(psf, ob):
        # quarter-granularity bias+store so the output DMAs overlap the
        # remaining ACT work in the epilogue
        for nn in range(2):
            ot = wk.tile([128, 512], f32, tag="ot", name=f"ot{ob}_{nn}", bufs=2)
            nc.scalar.activation(
                out=ot[:, :], in_=psf[:, nn * 512 : (nn + 1) * 512],
                func=AF.Identity, bias=sb_bo[:, ob : ob + 1],
            )
            nc.sync.dma_start(
                out=out_d[ob * 128 : (ob + 1) * 128, nn * 512 : (nn + 1) * 512],
                in_=ot[:, :],
            )

    def proj_t(t):
        """Output-projection partial for attention tile t, accumulated into
        SBUF f32 so the matmuls overlap attention instead of serializing at
        the tail.  t=0 evacuates via ACT with the bias fused; later t's add
        the PSUM partial on DVE; t=3 streams the result out."""
        for ob in range(2):
            psf = ppm.tile([128, S], f32, tag="mm", name=f"psf{ob}_{t}")
            for nn in range(2):
                nc.tensor.matmul(
                    psf[:, nn * 512 : (nn + 1) * 512],
                    lhsT=(sb_wo[:, t, ob * 128 : (ob + 1) * 128]),
                    rhs=(sb_o[:, t, nn * 512 : (nn + 1) * 512]),
                    start=True,
                    stop=True,
                )
            if t == 0:
                nc.scalar.activation(
                    out=acc[:, ob, :], in_=psf[:, :], func=AF.Identity,
                    bias=sb_bo[:, ob : ob + 1],
                )
            else:
                nc.vector.tensor_add(
                    out=acc[:, ob, :], in0=psf[:, :], in1=acc[:, ob, :]
                )
                if t == 3:
                    nc.sync.dma_start(
                        out=out_d[ob * 128 : (ob + 1) * 128, :], in_=acc[:, ob, :]
                    )

    # ---- emission order == scheduling priority ----
    qk_block(0)
    qk_block(4)
    rope_norm(0)
    for jb in range(8):
        v_block(jb)
    qk_block(1)
    qk_block(5)
    rope_norm(1)
    qk_block(2)
    qk_block(6)
    attn_head(0)
    rope_norm(2)
    attn_head(1)
    attn_norm(0)
    qk_block(3)
    qk_block(7)
    attn_head(2)
    attn_norm(1)
    rope_norm(3)
    attn_head(3)
    attn_norm(2)
    if TAILPROJ:
        attn_head(4)
        attn_norm(3)
        attn_head(5)
        attn_norm(4)
        attn_head(6)
        attn_norm(5)
        attn_head(7)
        # projection partials for the ready tiles run BEFORE the last two
        # normalize chains in the in-order PE queue; only kk=3 remains gated
        psf0 = ppm.tile([128, S], f32, tag="mm", name="psf0")
        psf1 = ppm.tile([128, S], f32, tag="mm", name="psf1")
        for kk in range(3):
            proj_kk(psf0, 0, kk)
            proj_kk(psf1, 1, kk)
        attn_norm(6)
        attn_norm(7)
        proj_kk(psf0, 0, 3)
        proj_kk(psf1, 1, 3)
        proj_out(psf0, 0)
        proj_out(psf1, 1)
    else:
        attn_head(4)
        attn_norm(3)
        attn_head(5)
        attn_norm(4)
        proj_t(0)
        attn_head(6)
        attn_norm(5)
        proj_t(1)
        attn_head(7)
        attn_norm(6)
        proj_t(2)
        attn_norm(7)
        proj_t(3)


def _patch_act_tables():
    """Steer the act-table-load pass to one set covering Exp+Ln+Copy.

    The default pass picks the first table set containing each activation
    function, which ping-pongs between exp_and_others and natural_log
    (~2.7us per reload).  Emptying every other set forces all activations
    onto natural_log_exp_and_others -> exactly one load.
    """
    import concourse.bacc as bacc

    if getattr(bacc, "_act_tables_patched", False):
        return
    import concourse.hw_specs as hw_specs

    orig = hw_specs.get_activation_tables

    def patched(arch):
        tables = orig(arch)
        keep = "natural_log_exp_and_others"
        assert keep in tables
        return {
            name: (fns if name == keep else set()) for name, fns in tables.items()
        }

    bacc.get_activation_tables = patched
    bacc._act_tables_patched = True


def _build():
    from contextlib import ExitStack

    import concourse.bacc as bacc
    import concourse.tile as tile
    from concourse import mybir

    _patch_act_tables()

    f32 = mybir.dt.float32
    bf16 = mybir.dt.bfloat16
    f8 = mybir.dt.float8e4
    nc = bacc.Bacc("TRN2", target_bir_lowering=False, debug=False, num_devices=N_CORES)
    aps = (
        nc.dram_tensor("x", [C, S], bf16, kind="ExternalInput").ap(),
        nc.dram_tensor("wqkT", [C, 2 * HID], bf16, kind="ExternalInput").ap(),
        nc.dram_tensor("x8", [128, 2, S], f8, kind="ExternalInput").ap(),
        nc.dram_tensor("wqk8", [128, 2, 2 * HID], f8, kind="ExternalInput").ap(),
        nc.dram_tensor("wvT", [C, HID], bf16, kind="ExternalInput").ap(),
        nc.dram_tensor("woT", [HID, C], bf16, kind="ExternalInput").ap(),
        nc.dram_tensor("bout", [128, 2], f32, kind="ExternalInput").ap(),
        nc.dram_tensor("cosT", [128, S], bf16, kind="ExternalInput").ap(),
        nc.dram_tensor("sinT", [128, S], bf16, kind="ExternalInput").ap(),
        nc.dram_tensor("out", [C, S], f32, kind="ExternalOutput").ap(),
    )
    with tile.TileContext(nc) as tc:
        with ExitStack() as ctx:
            _emit(ctx, tc, aps)
    nc.compile()
    return nc


def _get_nc():
    if "nc" not in _CACHE:
        _CACHE["nc"] = _build()
    return _CACHE["nc"]


def _make_in_maps(x, w_qkv, w_out, b_out):
    import ml_dtypes

    bf = ml_dtypes.bfloat16
    f8 = ml_dtypes.float8_e4m3
    xf32 = np.asarray(x, np.float32).reshape(N_CORES, C, S)
    xf = np.ascontiguousarray(xf32).astype(bf)
    wq = np.asarray(w_qkv, np.float32)
    wqkT = np.ascontiguousarray(wq[0 : 2 * HID, :].T).astype(bf)
    wvT = np.ascontiguousarray(wq[2 * HID : 3 * HID, :].T).astype(bf)
    if FP8:
        x8 = np.ascontiguousarray(
            np.stack([xf32[:, 0:128, :], xf32[:, 128:256, :]], axis=2)
        ).astype(f8)
        wqkT256 = wq[0 : 2 * HID, :].T * 256.0
        wqk8 = np.ascontiguousarray(
            np.stack([wqkT256[0:128, :], wqkT256[128:256, :]], axis=1)
        ).astype(f8)
    else:
        # declared inputs the bf16 kernel never reads: ship zeros, skip the cast
        x8 = np.zeros((N_CORES, 128, 2, S), f8)
        wqk8 = np.zeros((128, 2, 2 * HID), f8)
    woT = np.ascontiguousarray(np.asarray(w_out, np.float32).T).astype(bf)
    bo = np.ascontiguousarray(np.asarray(b_out, np.float32).reshape(2, 128).T)
    cosT, sinT = _rope_tables()
    shared = {
        "wqkT": wqkT,
        "wqk8": wqk8,
        "wvT": wvT,
        "woT": woT,
        "bout": bo,
        "cosT": cosT.astype(bf),
        "sinT": sinT.astype(bf),
    }
    return [
        dict(shared, x=np.ascontiguousarray(xf[i]), x8=np.ascontiguousarray(x8[i]))
        for i in range(N_CORES)
    ]


def _postprocess(res):
    out = np.stack([r["out"] for r in res.results], axis=0)
    return out.reshape(N_CORES, C, 32, 32).astype(np.float32)


def _run(x, w_qkv, w_out, b_out, trace=False):
    from concourse.bass_utils import run_bass_kernel_spmd

    nc = _get_nc()
    in_maps = _make_in_maps(x, w_qkv, w_out, b_out)
    res = run_bass_kernel_spmd(nc, in_maps, core_ids=list(range(N_CORES)), trace=trace)
    return _postprocess(res), res


def kernel(x, w_qkv, w_out, b_out):
    return _run(x, w_qkv, w_out, b_out, trace=False)[0]
